# revision 1
# baseline (speedup 1.0000x reference)
"""Trainium2 Bass kernel for nn_BatchProgramCC (tree-GRU program classifier).

Contract: kernel(**inputs) takes FULL unsharded inputs (numpy), returns the
FULL [64, 1] float32 output. Internally shards the B=64 program batch over
8 NeuronCores (8 programs / core), runs one SPMD Bass program, gathers.

Hardcoded problem shape (from the reference):
  V=30000 vocab, E=ENC=H=128, B=64, S=32 statements, K=64 nodes/tree,
  heap tree layout (parent of j is (j-1)//2 within each 64-node block),
  LEVELS=8 level-sync iterations == exact subtree sums (depth 6).
"""

import os
import numpy as np
from contextlib import ExitStack

try:
    import concourse.bass as bass
except ImportError:  # pragma: no cover
    import sys

    sys.path.insert(0, "/opt/trn_rl_repo")
    import concourse.bass as bass

import concourse.bacc as bacc
import concourse.mybir as mybir
import concourse.tile as tile
from concourse import masks
from concourse.bass_utils import run_bass_kernel_spmd

FP32 = mybir.dt.float32
FP16 = mybir.dt.float16
I16 = mybir.dt.int16
ALU = mybir.AluOpType
ACTF = mybir.ActivationFunctionType
AX = mybir.AxisListType

V = 30000
E = 128          # embedding/encode/hidden dim (all 128)
NCORES = 8
BL = 8           # programs per core
S = 32           # statements (GRU steps)
K = 64           # nodes per tree
TL = BL * S      # trees per side per core = 256
NL = TL * K      # nodes per side per core = 16384
GCH = 8192       # gather chunk (indices per dma_gather)
NGCH = NL // GCH  # = 2 chunks per side
STRIPES = (V + 127) // 128  # 235 table stripes
PROJ = 512       # projection matmul free dim (nodes per matmul)
_PHASE = int(os.environ.get("KPHASE", "99"))  # debug bisect knob
_KDBG = os.environ.get("KDBG", "")  # "", "h", "seq", "gat": extra debug output

# ---------------------------------------------------------------------------
# Tree level layout (level-major, recursive even/odd split ordering).
# sigma[l] = heap positions of level l, ordered so that the children of
# sigma[l][i] are sigma[l+1][i] (left) and sigma[l+1][i + n_l] (right).
# ---------------------------------------------------------------------------
_SIGMA = [[0]]
for _l in range(1, 6):
    _prev = _SIGMA[-1]
    _SIGMA.append([2 * p + 1 for p in _prev] + [2 * p + 2 for p in _prev])
_SIGMA.append([63])  # level 6: only heap node 63 (= left child of 31)
_NLVL = [len(s) for s in _SIGMA]            # [1, 2, 4, 8, 16, 32, 1]
_OFF = [0]
for _n in _NLVL:
    _OFF.append(_OFF[-1] + TL * _n)          # level block column offsets
assert _OFF[-1] == NL


def _node_perm() -> np.ndarray:
    """perm[col] = within-core node index (tree*64 + heap_j) for stored col."""
    order = []
    for lvl in range(7):
        for t in range(TL):
            for hp in _SIGMA[lvl]:
                order.append(t * K + hp)
    return np.asarray(order, dtype=np.int64)


_PERM = _node_perm()


def _wrap_idx(tokens_perm: np.ndarray) -> np.ndarray:
    """Wrap permuted int token list [NL] into dma_gather idx layout
    [NGCH, 128, GCH//16] int16 (idx j of a chunk -> row j%16, col j//16)."""
    out = np.zeros((NGCH, 128, GCH // 16), dtype=np.int16)
    for c in range(NGCH):
        chunk = tokens_perm[c * GCH : (c + 1) * GCH].astype(np.int16)
        blk = chunk.reshape(GCH // 16, 16).T
        for grp in range(8):  # replicated per Q7 core group (HW reads all 8)
            out[c, grp * 16 : (grp + 1) * 16, :] = blk
    return out


# ---------------------------------------------------------------------------
# Kernel builder
# ---------------------------------------------------------------------------

def build_nc() -> bass.Bass:
    nc = bacc.Bacc("TRN2", target_bir_lowering=False)

    tok = [
        nc.declare_dram_parameter("tok1", [NGCH, 128, GCH // 16], I16, isOutput=False),
        nc.declare_dram_parameter("tok2", [NGCH, 128, GCH // 16], I16, isOutput=False),
    ]
    emb = nc.declare_dram_parameter("emb", [V, E], FP32, isOutput=False)
    w_c = nc.declare_dram_parameter("w_c", [E, E], FP32, isOutput=False)
    b_c = nc.declare_dram_parameter("b_c", [E], FP32, isOutput=False)
    w_ih = {
        "f": nc.declare_dram_parameter("w_ih_f", [3 * E, E], FP32, isOutput=False),
        "b": nc.declare_dram_parameter("w_ih_b", [3 * E, E], FP32, isOutput=False),
    }
    w_hh = {
        "f": nc.declare_dram_parameter("w_hh_f", [3 * E, E], FP32, isOutput=False),
        "b": nc.declare_dram_parameter("w_hh_b", [3 * E, E], FP32, isOutput=False),
    }
    b_ih = {
        "f": nc.declare_dram_parameter("b_ih_f", [3 * E], FP32, isOutput=False),
        "b": nc.declare_dram_parameter("b_ih_b", [3 * E], FP32, isOutput=False),
    }
    b_hh = {
        "f": nc.declare_dram_parameter("b_hh_f", [3 * E], FP32, isOutput=False),
        "b": nc.declare_dram_parameter("b_hh_b", [3 * E], FP32, isOutput=False),
    }
    w_out = nc.declare_dram_parameter("w_out", [1, E], FP32, isOutput=False)
    b_out = nc.declare_dram_parameter("b_out", [1], FP32, isOutput=False)
    out_ext = nc.declare_dram_parameter("out", [BL], FP32, isOutput=True)

    with tile.TileContext(nc) as tc, ExitStack() as ctx:
        persist = ctx.enter_context(tc.tile_pool(name="persist", bufs=1))
        work = ctx.enter_context(tc.tile_pool(name="work", bufs=1))

        # ---- constant / weight prep -------------------------------------
        ident = persist.tile([128, 128], FP32)
        masks.make_identity(nc, ident[:])

        # fp16 token table: token i -> partition i%128, elems [128*(i//128) ..)
        table = persist.tile([128, STRIPES * 128], FP16)
        main_s = (V // 128)  # 234 full stripes
        nsplit = 3  # stay under the 16384-descriptor-per-DMA cap
        per = (main_s + nsplit - 1) // nsplit
        for sp in range(nsplit):
            s0, s1 = sp * per, min((sp + 1) * per, main_s)
            nc.gpsimd.dma_start(
                out=table[:, s0 * 128 : s1 * 128].rearrange("p (s e) -> p s e", e=128),
                in_=emb[s0 * 128 : s1 * 128].rearrange("(s p) e -> p s e", p=128),
                single_packet=False,
            )
        tail = V - main_s * 128  # 48
        nc.gpsimd.memset(table[:, main_s * 128 : (main_s + 1) * 128], 0.0)
        nc.gpsimd.dma_start(
            out=table[0:tail, main_s * 128 : (main_s + 1) * 128],
            in_=emb[main_s * 128 : V],
        )

        # biases to per-partition layout
        bc_col = persist.tile([128, 1], FP32)
        nc.sync.dma_start(out=bc_col[:], in_=b_c.rearrange("(p o) -> p o", o=1))
        bih_sb = {}
        bhh_sb = {}
        bsum_sb = {}
        for d in ("f", "b"):
            tih = persist.tile([128, 3], FP32, name=f"bih_{d}")
            nc.sync.dma_start(out=tih[:], in_=b_ih[d].rearrange("(g p) -> p g", g=3))
            thh = persist.tile([128, 3], FP32, name=f"bhh_{d}")
            nc.sync.dma_start(out=thh[:], in_=b_hh[d].rearrange("(g p) -> p g", g=3))
            tsum = persist.tile([128, 2], FP32, name=f"bsum_{d}")
            nc.vector.tensor_tensor(
                out=tsum[:], in0=tih[:, 0:2], in1=thh[:, 0:2], op=ALU.add
            )
            bih_sb[d], bhh_sb[d], bsum_sb[d] = tih, thh, tsum

        wout_col = persist.tile([128, 1], FP32)
        nc.sync.dma_start(out=wout_col[:], in_=w_out.rearrange("o p -> p o"))
        bout_sb = persist.tile([1, 1], FP32)
        nc.sync.dma_start(out=bout_sb[:], in_=b_out.rearrange("(p o) -> p o", o=1))

        # transposed weights via PE
        wcT = persist.tile([128, 128], FP16)  # [e, c] fp16
        wihT = {d: persist.tile([128, 3 * E], FP32, name=f"wihT_{d}") for d in ("f", "b")}
        whhT = {d: persist.tile([128, 3 * E], FP32, name=f"whhT_{d}") for d in ("f", "b")}
        with tc.tile_pool(name="wload", bufs=3) as wload, tc.tile_pool(
            name="tpsum", bufs=2, space="PSUM"
        ) as tpsum:
            wc_raw = wload.tile([128, 128], FP32, tag="wraw")
            nc.sync.dma_start(out=wc_raw[:], in_=w_c[:, :])
            pt = tpsum.tile([128, 128], FP32, tag="tp")
            nc.tensor.transpose(pt[:], wc_raw[:], ident[:])
            nc.vector.tensor_copy(out=wcT[:], in_=pt[:])
            for d in ("f", "b"):
                for g in range(3):
                    raw = wload.tile([128, 128], FP32, tag="wraw")
                    nc.sync.dma_start(out=raw[:], in_=w_ih[d][g * 128 : (g + 1) * 128, :])
                    pt = tpsum.tile([128, 128], FP32, tag="tp")
                    nc.tensor.transpose(pt[:], raw[:], ident[:])
                    nc.vector.tensor_copy(out=wihT[d][:, g * 128 : (g + 1) * 128], in_=pt[:])
                    raw = wload.tile([128, 128], FP32, tag="wraw")
                    nc.sync.dma_start(out=raw[:], in_=w_hh[d][g * 128 : (g + 1) * 128, :])
                    pt = tpsum.tile([128, 128], FP32, tag="tp")
                    nc.tensor.transpose(pt[:], raw[:], ident[:])
                    nc.vector.tensor_copy(out=whhT[d][:, g * 128 : (g + 1) * 128], in_=pt[:])

        # idx tiles
        idx_sb = []
        for side in range(2):
            per = []
            for c in range(NGCH):
                it = persist.tile([128, GCH // 16], I16, name=f"idx{side}{c}")
                nc.sync.dma_start(out=it[:], in_=tok[side][c])
                per.append(it)
            idx_sb.append(per)

        # seq: [128, 512] fp32, col = stmt*16 + side*8 + prog
        seq = persist.tile([128, 2 * TL], FP32)

        # ---- per-side: gather -> project -> tree sums -> tree max -------
        h = persist.tile([128, NL], FP16)
        tmp = work.tile([128, TL * 16], FP16, tag="tmp_add")
        mx = [
            work.tile([128, TL * n], FP16, name=f"mx{n}")
            for n in (16, 8, 4, 2, 1)
        ]

        with tc.tile_pool(name="gat", bufs=2) as gat, tc.tile_pool(
            name="ppsum", bufs=3, space="PSUM"
        ) as ppsum:
            for side in range(2 if _PHASE >= 2 else 0):
                gts = []
                for c in range(NGCH):
                    gt = gat.tile([128, GCH], FP16, tag="g")
                    nc.gpsimd.dma_gather(
                        gt[:].rearrange("p (o n) -> p o n", o=1),
                        table[:],
                        idx_sb[side][c][:],
                        GCH,
                        GCH,
                        E,
                        transpose=True,
                        single_packet=False,
                        sbuf_tokens_per_rank=128,
                        sbuf_free_dim_per_rank=256,
                    )
                    gts.append(gt)
                # projection: psum[c, nodes] = w_c @ emb_nodes ; +b_c -> h fp16
                for m in range(NL // PROJ if _PHASE >= 3 else 0):
                    gt = gts[m * PROJ // GCH]
                    rhs = gt[:, (m * PROJ) % GCH : (m * PROJ) % GCH + PROJ]
                    ps = ppsum.tile([128, PROJ], FP32, tag="proj")
                    nc.tensor.matmul(ps[:], wcT[:], rhs, start=True, stop=True)
                    dst = h[:, m * PROJ : (m + 1) * PROJ]
                    if m % 3 == 2:
                        nc.scalar.activation(dst, ps[:], ACTF.Identity, bias=bc_col[:])
                    else:
                        nc.vector.tensor_scalar(
                            out=dst, in0=ps[:], scalar1=bc_col[:], scalar2=None,
                            op0=ALU.add,
                        )

                if _KDBG == "gat" and side == 0:
                    dbg_g = nc.declare_dram_parameter("dbg", [128, NL], FP16, isOutput=True)
                    for c in range(NGCH):
                        nc.sync.dma_start(
                            out=dbg_g[:, c * GCH : (c + 1) * GCH], in_=gts[c][:]
                        )
                # bottom-up subtree sums (contiguous fp16 ops)
                if _PHASE < 4:
                    continue
                # L6: h[L5 block, idx0 of each tree] += h[L6 block]
                nc.vector.tensor_tensor(
                    out=h[:, _OFF[5] : _OFF[5] + TL * 32]
                    .rearrange("p (t n) -> p t n", n=32)[:, :, 0:1],
                    in0=h[:, _OFF[5] : _OFF[5] + TL * 32]
                    .rearrange("p (t n) -> p t n", n=32)[:, :, 0:1],
                    in1=h[:, _OFF[6] : _OFF[6] + TL].rearrange(
                        "p (t n) -> p t n", n=1
                    ),
                    op=ALU.add,
                )
                for lvl in range(4, -1, -1):
                    nl_, nl2 = _NLVL[lvl], 2 * _NLVL[lvl]
                    fd = TL * nl_
                    child = h[:, _OFF[lvl + 1] : _OFF[lvl + 1] + TL * nl2].rearrange(
                        "p (t n) -> p t n", n=nl2
                    )
                    tview = tmp[:, 0:fd].rearrange("p (t n) -> p t n", n=nl_)
                    nc.vector.tensor_tensor(
                        out=tview, in0=child[:, :, 0:nl_], in1=child[:, :, nl_:nl2],
                        op=ALU.add,
                    )
                    nc.vector.tensor_tensor(
                        out=h[:, _OFF[lvl] : _OFF[lvl] + fd],
                        in0=h[:, _OFF[lvl] : _OFF[lvl] + fd],
                        in1=tmp[:, 0:fd],
                        op=ALU.add,
                    )

                if _KDBG == "h" and side == 0:
                    dbg_h = nc.declare_dram_parameter("dbg", [128, NL], FP16, isOutput=True)
                    nc.sync.dma_start(out=dbg_h[:, :], in_=h[:])
                # max over the 64 per-node subtree sums, per tree
                if _PHASE < 5:
                    continue
                def halve(dst, src, n):
                    sv = src.rearrange("p (t n) -> p t n", n=n)
                    nc.vector.tensor_tensor(
                        out=dst.rearrange("p (t n) -> p t n", n=n // 2),
                        in0=sv[:, :, 0 : n // 2], in1=sv[:, :, n // 2 : n],
                        op=ALU.max,
                    )

                def fold(dst, blk_lvl):
                    nc.vector.tensor_tensor(
                        out=dst, in0=dst,
                        in1=h[:, _OFF[blk_lvl] : _OFF[blk_lvl] + dst.shape[1]],
                        op=ALU.max,
                    )

                halve(mx[0][:], h[:, _OFF[5] : _OFF[5] + TL * 32], 32)  # L5 -> 16
                fold(mx[0][:], 4)
                halve(mx[1][:], mx[0][:], 16)
                fold(mx[1][:], 3)
                halve(mx[2][:], mx[1][:], 8)
                fold(mx[2][:], 2)
                halve(mx[3][:], mx[2][:], 4)
                fold(mx[3][:], 1)
                halve(mx[4][:], mx[3][:], 2)
                fold(mx[4][:], 0)
                fold(mx[4][:], 6)

                # relu + scatter into seq (col = stmt*16 + side*8 + prog)
                nc.vector.tensor_scalar(
                    out=seq[:].rearrange("p (k e r) -> p r e k", e=2, r=BL)[:, :, side, :],
                    in0=mx[4][:].rearrange("p (r k) -> p r k", k=S),
                    scalar1=0.0, scalar2=None, op0=ALU.max,
                )

        if _KDBG == "seq":
            dbg_s = nc.declare_dram_parameter("dbg", [128, 2 * TL], FP32, isOutput=True)
            nc.sync.dma_start(out=dbg_s[:, :], in_=seq[:])

        # ---- GRU (fwd + bwd, both sides batched; 16 lanes per dir) ------
        gin = persist.tile([128, S * 32], FP32)  # col = k*32 + dir*16 + sp
        h_all = persist.tile([128, 32], FP32)    # col = dir*16 + side*8 + prog
        rz_all = persist.tile([128, 64], FP32)   # col = dir*32 + gate*16 + sp
        n_all = persist.tile([128, 32], FP32)
        nc.vector.memset(h_all[:], 0.0)

        dirs = ("f", "b")
        girz_sb = persist.tile([128, S * 64], FP32)  # col = k*64+g*32+d*16+sp
        with tc.tile_pool(
            name="ghrz", bufs=2, space="PSUM"
        ) as ghrzp, tc.tile_pool(name="gsmall", bufs=3) as gsmall:
            girz_r = girz_sb[:].rearrange("p (k g d j) -> p k g d j", g=2, d=2, j=16)
            gin_r = gin[:].rearrange("p (k d j) -> p k d j", d=2, j=16)

            with tc.tile_pool(name="gnpsum", bufs=2, space="PSUM") as gnpsum:
                for di, d in (list(enumerate(dirs)) if _PHASE >= 6 else []):
                    for g in range(2):  # r, z -> SBUF Gi (+ summed biases)
                        pg = gnpsum.tile([128, 512], FP32, tag="gn")
                        nc.tensor.matmul(
                            pg[:], wihT[d][:, g * 128 : (g + 1) * 128], seq[:],
                            start=True, stop=True,
                        )
                        nc.vector.tensor_scalar(
                            out=girz_r[:, :, g, di, :],
                            in0=pg[:].rearrange("p (k j) -> p k j", j=16),
                            scalar1=bsum_sb[d][:, g : g + 1], scalar2=None,
                            op0=ALU.add,
                        )
                    # n-gate: psum -> SBUF with b_ih_n bias
                    for hh in range(2):
                        pn = gnpsum.tile([128, 256], FP32, tag="gn")
                        nc.tensor.matmul(
                            pn[:], wihT[d][:, 256:384],
                            seq[:, hh * 256 : (hh + 1) * 256], start=True, stop=True,
                        )
                        nc.scalar.activation(
                            gin_r[:, hh * 16 : (hh + 1) * 16, di, :],
                            pn[:].rearrange("p (k j) -> p k j", j=16),
                            ACTF.Identity, bias=bih_sb[d][:, 2:3],
                        )

            for k in range(S if _PHASE >= 7 else 0):
                for di, d in enumerate(dirs):
                    td = k if d == "f" else S - 1 - k
                    hsl = h_all[:, di * 16 : (di + 1) * 16]
                    ghrz = ghrzp.tile([128, 48], FP32, tag=f"ghrz{di}")
                    for g in range(3):
                        nc.tensor.matmul(
                            ghrz[:, g * 16 : (g + 1) * 16],
                            whhT[d][:, g * 128 : (g + 1) * 128],
                            hsl, start=True, stop=True,
                        )
                    ghn = ghrz[:, 32:48]
                    rzin = gsmall.tile([128, 32], FP32, tag=f"rzin{di}")
                    nc.vector.tensor_tensor(
                        out=rzin[:].rearrange("p (g j) -> p g j", g=2),
                        in0=girz_r[:, td, :, di, :],
                        in1=ghrz[:, 0:32].rearrange("p (g j) -> p g j", g=2),
                        op=ALU.add,
                    )
                    nc.scalar.activation(
                        rz_all[:, di * 32 : (di + 1) * 32], rzin[:], ACTF.Sigmoid
                    )
                    nmul = gsmall.tile([128, 16], FP32, tag=f"nmul{di}")
                    nc.vector.scalar_tensor_tensor(
                        out=nmul[:], in0=ghn, scalar=bhh_sb[d][:, 2:3],
                        in1=rz_all[:, di * 32 : di * 32 + 16],
                        op0=ALU.add, op1=ALU.mult,
                    )
                    ninp = gsmall.tile([128, 16], FP32, tag=f"ninp{di}")
                    nc.vector.tensor_tensor(
                        out=ninp[:], in0=gin_r[:, td, di, :], in1=nmul[:], op=ALU.add
                    )
                    nsl = n_all[:, di * 16 : (di + 1) * 16]
                    nc.scalar.activation(nsl, ninp[:], ACTF.Tanh)
                    # h = n + z * (h - n)   (per dir: chains stay independent)
                    t1 = gsmall.tile([128, 16], FP32, tag=f"bl1{di}")
                    nc.vector.tensor_tensor(
                        out=t1[:], in0=hsl, in1=nsl, op=ALU.subtract
                    )
                    t2 = gsmall.tile([128, 16], FP32, tag=f"bl2{di}")
                    nc.vector.tensor_tensor(
                        out=t2[:], in0=t1[:],
                        in1=rz_all[:, di * 32 + 16 : di * 32 + 32], op=ALU.mult,
                    )
                    nc.vector.tensor_tensor(
                        out=hsl, in0=nsl, in1=t2[:], op=ALU.add
                    )

            # ---- head: sigmoid(|l - r| @ w_out.T + b_out) ----------------
            if _PHASE >= 8:
                hs = gsmall.tile([128, 16], FP32, tag="hs")
                nc.vector.tensor_tensor(
                    out=hs[:], in0=h_all[:, 0:16], in1=h_all[:, 16:32], op=ALU.add
                )
                d0 = gsmall.tile([128, 8], FP32, tag="d0")
                nc.vector.tensor_tensor(
                    out=d0[:], in0=hs[:, 0:8], in1=hs[:, 8:16], op=ALU.subtract
                )
                dabs = gsmall.tile([128, 8], FP32, tag="dabs")
                nc.scalar.activation(dabs[:], d0[:], ACTF.Abs)
                with tc.tile_pool(name="opsum", bufs=1, space="PSUM") as opsum:
                    po = opsum.tile([1, 8], FP32)
                    nc.tensor.matmul(po[:], wout_col[:], dabs[:], start=True, stop=True)
                    osb = gsmall.tile([1, 8], FP32, tag="osb")
                    nc.scalar.activation(osb[:], po[:], ACTF.Sigmoid, bias=bout_sb[:])
                    nc.sync.dma_start(
                        out=out_ext.rearrange("(o j) -> o j", o=1), in_=osb[:]
                    )
            else:
                osb0 = gsmall.tile([1, 8], FP32, tag="osb0")
                nc.vector.memset(osb0[:], 0.5)
                nc.sync.dma_start(
                    out=out_ext.rearrange("(o j) -> o j", o=1), in_=osb0[:]
                )

    # full legalization: wait splitting, gpsimd library-load insertion,
    # extended-inst ISA codegen, reg alloc.
    nc.compile()
    return nc


_NC_CACHE = None


def _get_nc():
    global _NC_CACHE
    if _NC_CACHE is None:
        _NC_CACHE = build_nc()
    return _NC_CACHE


def make_in_maps(inputs: dict) -> list:
    """Host-side prep: shard + permute tokens, replicate weights."""
    tokens1 = np.asarray(inputs["tokens1"]).astype(np.int64)
    tokens2 = np.asarray(inputs["tokens2"]).astype(np.int64)
    rep = {
        "emb": np.ascontiguousarray(np.asarray(inputs["emb"], np.float32)),
        "w_c": np.asarray(inputs["w_c"], np.float32),
        "b_c": np.asarray(inputs["b_c"], np.float32),
        "w_ih_f": np.asarray(inputs["w_ih_f"], np.float32),
        "w_hh_f": np.asarray(inputs["w_hh_f"], np.float32),
        "b_ih_f": np.asarray(inputs["b_ih_f"], np.float32),
        "b_hh_f": np.asarray(inputs["b_hh_f"], np.float32),
        "w_ih_b": np.asarray(inputs["w_ih_b"], np.float32),
        "w_hh_b": np.asarray(inputs["w_hh_b"], np.float32),
        "b_ih_b": np.asarray(inputs["b_ih_b"], np.float32),
        "b_hh_b": np.asarray(inputs["b_hh_b"], np.float32),
        "w_out": np.asarray(inputs["w_out"], np.float32),
        "b_out": np.asarray(inputs["b_out"], np.float32),
    }
    in_maps = []
    for i in range(NCORES):
        t1 = tokens1[i * NL : (i + 1) * NL][_PERM]
        t2 = tokens2[i * NL : (i + 1) * NL][_PERM]
        in_maps.append({"tok1": _wrap_idx(t1), "tok2": _wrap_idx(t2), **rep})
    return in_maps


def kernel(**inputs) -> np.ndarray:
    nc = _get_nc()
    in_maps = make_in_maps(inputs)
    res = run_bass_kernel_spmd(nc, in_maps, list(range(NCORES)))
    out = np.concatenate(
        [np.asarray(res.results[i]["out"], np.float32).reshape(BL, 1) for i in range(NCORES)],
        axis=0,
    )
    return out



# revision 28
# speedup vs baseline: 2.4869x; 2.4869x over previous
"""Trainium2 Bass kernel for nn_BatchProgramCC (tree-GRU program classifier).

Contract: kernel(**inputs) takes FULL unsharded inputs (numpy), returns the
FULL [64, 1] float32 output. Internally shards the B=64 program batch over
8 NeuronCores (8 programs / core), runs one SPMD Bass program, gathers.

Hardcoded problem shape (from the reference):
  V=30000 vocab, E=ENC=H=128, B=64, S=32 statements, K=64 nodes/tree,
  heap tree layout (parent of j is (j-1)//2 within each 64-node block),
  LEVELS=8 level-sync iterations == exact subtree sums (depth 6).

Pipeline per core (per side, 16384 nodes):
  - transpose-mode dma_gather straight from the fp16 embedding table in
    DRAM -> gT[E, node] in SBUF, node columns in level-major order,
    split into 5 two-ended statement groups (pair counts 2,4,4,4,2) so
    the fused fwd+bwd GRU can start early and finish soon after the
    last gather.
  - level-synchronous subtree sums in-place on gT (fp16; pair-sums on
    Pool, accumulate on DVE).
  - projection W_c @ S per <=512-col chunk on PE (fp16), + b_c x size
    rank-1 matmul into the same PSUM; max-folding reads PSUM once
    (Act seeds + DVE folds; GPSIMD cannot touch PSUM on HW).
  - relu + scatter into per-(step, dir) GRU input layout (Pool).
  - fused fwd+bwd GRU, 4 gates (r, z, 1-z, n) so the blend is
    h' = (1-z)*n + z*h with two short tensor ops after tanh.
"""

import numpy as np
from contextlib import ExitStack

try:
    import concourse.bass as bass
except ImportError:  # pragma: no cover
    import sys

    sys.path.insert(0, "/opt/trn_rl_repo")
    import concourse.bass as bass

import concourse.bacc as bacc
import concourse.mybir as mybir
import concourse.tile as tile
from concourse import masks
from concourse.bass_utils import run_bass_kernel_spmd

FP32 = mybir.dt.float32
FP16 = mybir.dt.float16
I16 = mybir.dt.int16
ALU = mybir.AluOpType
ACTF = mybir.ActivationFunctionType

V = 30000
E = 128          # embedding/encode/hidden dim (all 128)
NCORES = 8
BL = 8           # programs per core
S = 32           # statements (GRU steps)
K = 64           # nodes per tree
NL = BL * S * K  # nodes per side per core = 16384

PAIRC = [2, 4, 4, 4, 2]          # statement pairs per two-ended group
NGRP = len(PAIRC)
SG = [0] * NGRP                  # first step of each group
for _g in range(1, NGRP):
    SG[_g] = SG[_g - 1] + PAIRC[_g - 1]
GTREES = [2 * c * BL for c in PAIRC]   # trees per (group, side)
GNODE = [t * K for t in GTREES]        # nodes per (group, side)
GBASE = [0] * NGRP
for _g in range(1, NGRP):
    GBASE[_g] = GBASE[_g - 1] + GNODE[_g - 1]
assert GBASE[-1] + GNODE[-1] == NL

# ---------------------------------------------------------------------------
# Tree level layout (level-major, recursive even/odd split ordering).
# sigma[l] = heap positions of level l, ordered so that the children of
# sigma[l][i] are sigma[l+1][i] (left) and sigma[l+1][i + n_l] (right).
# ---------------------------------------------------------------------------
_SIGMA = [[0]]
for _l in range(1, 6):
    _prev = _SIGMA[-1]
    _SIGMA.append([2 * p + 1 for p in _prev] + [2 * p + 2 for p in _prev])
_SIGMA.append([63])  # level 6: only heap node 63 (= left child of 31)
_NLVL = [len(s) for s in _SIGMA]            # [1, 2, 4, 8, 16, 32, 1]


def _offsets(gt: int) -> list:
    off = [0]
    for n in _NLVL:
        off.append(off[-1] + gt * n)
    return off


_OFFS = [_offsets(t) for t in GTREES]

# subtree sizes per heap position
_SZS = [0] * K
for _j in range(K - 1, -1, -1):
    _SZS[_j] = 1
    if 2 * _j + 1 < K:
        _SZS[_j] += _SZS[2 * _j + 1]
    if 2 * _j + 2 < K:
        _SZS[_j] += _SZS[2 * _j + 2]


def _group_stmts(g: int) -> list:
    lo = [SG[g] + i for i in range(PAIRC[g])]
    hi0 = S - SG[g] - PAIRC[g]
    hi = [hi0 + i for i in range(PAIRC[g])]
    return lo + hi


def _group_perm(g: int) -> np.ndarray:
    """perm[col] = (side, within-core node index) for level-major col of
    group g; sides interleaved at the lane level (t_local = si*16+side*8+p).
    Encoded as side * NL + node."""
    stmts = _group_stmts(g)
    order = []
    for lvl in range(7):
        for t_local in range(2 * GTREES[g]):
            si, side, p = t_local // 16, (t_local // 8) % 2, t_local % 8
            tree_global = p * S + stmts[si]
            for hp in _SIGMA[lvl]:
                order.append(side * NL + tree_global * K + hp)
    return np.asarray(order, dtype=np.int64)


_PERMS = [_group_perm(g) for g in range(NGRP)]


def _szs_levelmajor(gt: int) -> np.ndarray:
    return np.asarray(
        [_SZS[hp] for lvl in range(7) for _t in range(gt) for hp in _SIGMA[lvl]],
        dtype=np.float16,
    )


def _wrap_idx(tokens_perm: np.ndarray) -> np.ndarray:
    """Wrap permuted token list into dma_gather idx layout
    [128, n//16] int16 (idx j -> row j%16, col j//16, replicated x8)."""
    n = len(tokens_perm)
    out = np.zeros((128, n // 16), dtype=np.int16)
    blk = tokens_perm.reshape(n // 16, 16).T.astype(np.int16)
    for grp in range(8):
        out[grp * 16 : (grp + 1) * 16, :] = blk
    return out


# ---------------------------------------------------------------------------
# Kernel builder
# ---------------------------------------------------------------------------

def build_nc() -> bass.Bass:
    nc = bacc.Bacc("TRN2", target_bir_lowering=False)

    emb16 = nc.declare_dram_parameter("emb16", [V, E], FP16, isOutput=False)
    idx_ext = [
        nc.declare_dram_parameter(f"idx{g}", [128, 2 * GNODE[g] // 16], I16, isOutput=False)
        for g in range(NGRP)
    ]
    wcT_ext = nc.declare_dram_parameter("wcT", [E, E], FP16, isOutput=False)
    wih_ext = nc.declare_dram_parameter("wih", [2, E, 3 * E], FP16, isOutput=False)
    whh_ext = nc.declare_dram_parameter("whh", [2, E, 3 * E], FP16, isOutput=False)
    bgi_ext = nc.declare_dram_parameter("bgi", [2, 1, 3 * E], FP16, isOutput=False)
    ghcst_ext = nc.declare_dram_parameter("ghcst", [E, S * 32], FP16, isOutput=False)
    wout_ext = nc.declare_dram_parameter("wout", [E, 1], FP16, isOutput=False)
    bout_ext = nc.declare_dram_parameter("bout", [1, 1], FP32, isOutput=False)
    out_ext = nc.declare_dram_parameter("out", [BL], FP32, isOutput=True)

    with tile.TileContext(nc) as tc, ExitStack() as ctx:
        persist = ctx.enter_context(tc.tile_pool(name="persist", bufs=1))
        work = ctx.enter_context(tc.tile_pool(name="work", bufs=2))

        # ---- prologue: constants + weights -----------------------------
        ident = persist.tile([128, 128], FP16)
        masks.make_identity(nc, ident[:])
        ones = persist.tile([1, 64], FP16)
        nc.vector.memset(ones[:], 1.0)

        idx_sb = []
        for i in range(NGRP):
            it = persist.tile([128, 2 * GNODE[i] // 16], I16, name=f"idx{i}")
            nc.sync.dma_start(out=it[:], in_=idx_ext[i][:, :])
            idx_sb.append(it)

        wcT = persist.tile([E, E], FP16)
        nc.sync.dma_start(out=wcT[:], in_=wcT_ext[:, :])
        wih = {}
        whh = {}
        bgi = {}
        for di, d in enumerate(("f", "b")):
            wih[d] = persist.tile([E, 3 * E], FP16, name=f"wih_{d}")
            nc.sync.dma_start(out=wih[d][:], in_=wih_ext[di])
            whh[d] = persist.tile([E, 3 * E], FP16, name=f"whh_{d}")
            nc.sync.dma_start(out=whh[d][:], in_=whh_ext[di])
            bgi[d] = persist.tile([1, 3 * E], FP16, name=f"bgi_{d}")
            nc.sync.dma_start(out=bgi[d][:], in_=bgi_ext[di])
        wout = persist.tile([E, 1], FP16)
        nc.sync.dma_start(out=wout[:], in_=wout_ext[:, :])
        bout = persist.tile([1, 1], FP32)
        nc.sync.dma_start(out=bout[:], in_=bout_ext[:, :])

        # GRU state + precomputed-input buffers
        # girz step block (96 cols): [rf rb znf znb ghnf ghnb]
        girz = persist.tile([128, S * 96], FP16)
        girz_r = girz[:].rearrange("p (k c) -> p k c", c=96)
        nc.sync.dma_start(
            out=girz_r[:, :, 64:96],
            in_=ghcst_ext.rearrange("p (k c) -> p k c", c=32),
        )
        gin = persist.tile([128, S * 32], FP16)   # step block: nf(16) nb(16)
        gin_r = gin[:].rearrange("p (k c) -> p k c", c=32)
        seq2 = persist.tile([128, S * 32], FP16)  # col = k*32 + d*16 + side*8 + prog
        seq2_r = seq2[:].rearrange("p (k d l) -> p k d l", d=2, l=16)
        h_all = persist.tile([128, 32], FP16)     # [hf(16) hb(16)], lane = side*8+prog
        nc.vector.memset(h_all[:], 0.0)

        gT = persist.tile([128, 2 * NL], FP16)

        ppool = ctx.enter_context(tc.tile_pool(name="proj", bufs=3, space="PSUM"))
        gipool = ctx.enter_context(tc.tile_pool(name="gip", bufs=2, space="PSUM"))
        sppool = ctx.enter_context(tc.tile_pool(name="stepp", bufs=2, space="PSUM"))

        # ---------------- tree phase for one (group, side) ----------------
        def gather_group(g: int):
            n = 2 * GNODE[g]
            reg = gT[:, 2 * GBASE[g] : 2 * GBASE[g] + n]
            gv = reg.rearrange("p (o n) -> p o n", o=1)
            nc.gpsimd.dma_gather(
                gv, emb16[:, :], idx_sb[g][:], n, n, E,
                transpose=True, single_packet=False,
            )

        def tree_group(g: int):
            OFF = _OFFS[g]
            GT = 2 * GTREES[g]
            reg = gT[:, 2 * GBASE[g] : 2 * GBASE[g] + 2 * GNODE[g]]

            # subtree sums, level-synchronous, in place (fp16, SBUF only)
            seng = nc.gpsimd if g >= 2 else nc.vector
            l5 = reg[:, OFF[5] : OFF[6]].rearrange("p (t n) -> p t n", n=32)
            seng.tensor_tensor(
                out=l5[:, :, 0:1], in0=l5[:, :, 0:1],
                in1=reg[:, OFF[6] : OFF[7]].rearrange("p (t n) -> p t n", n=1),
                op=ALU.add,
            )
            tmp = work.tile([128, 2 * GTREES[1] * 16], FP16, tag="tmp")
            for lvl in range(4, -1, -1):
                n_l = _NLVL[lvl]
                fd = GT * n_l
                child = reg[:, OFF[lvl + 1] : OFF[lvl + 2]].rearrange(
                    "p (t n) -> p t n", n=2 * n_l
                )
                tv = tmp[:, 0:fd].rearrange("p (t n) -> p t n", n=n_l)
                seng.tensor_tensor(
                    out=tv, in0=child[:, :, 0:n_l], in1=child[:, :, n_l : 2 * n_l],
                    op=ALU.add,
                )
                seng.tensor_tensor(
                    out=reg[:, OFF[lvl] : OFF[lvl] + fd],
                    in0=reg[:, OFF[lvl] : OFF[lvl] + fd],
                    in1=tmp[:, 0:fd],
                    op=ALU.add,
                )

            # projection chunks + bias + max folds (PSUM read once:
            # Act seeds the L5 left halves, DVE does all PSUM max-folds)
            def proj_chunk(ranges, mmax=512):
                width = sum(b - a for a, b in ranges)
                ps = ppool.tile([128, width], FP32, tag="proj")
                c0 = 0
                for a, b in ranges:
                    while a < b:
                        w = min(b - a, mmax)
                        nc.tensor.matmul(
                            ps[:, c0 : c0 + w], wcT[:], reg[:, a : a + w],
                            start=True, stop=True,
                        )
                        a += w
                        c0 += w
                return ps

            mx0 = work.tile([128, 2 * GTREES[1] * 16], FP16, tag="mx0")
            mx1 = work.tile([128, 2 * GTREES[1] * 8], FP16, tag="mx1")
            mx2 = work.tile([128, 2 * GTREES[1] * 4], FP16, tag="mx2")
            mx3 = work.tile([128, 2 * GTREES[1] * 2], FP16, tag="mx3")
            mx4 = work.tile([128, 2 * GTREES[1]], FP16, tag="mx4")

            veng = nc.gpsimd if g >= 2 else nc.vector

            def halve(dst, src, n):
                nc.vector.tensor_tensor(
                    out=dst.rearrange("p (t n) -> p t n", n=n // 2),
                    in0=src[:, :, 0 : n // 2], in1=src[:, :, n // 2 : n],
                    op=ALU.max,
                )

            # L5 -> mx0 (chunks of <=512)
            nl5 = GT * 32 // 512
            for c in range(nl5):
                ps = proj_chunk([(OFF[5] + c * 512, OFF[5] + (c + 1) * 512)])
                pv = ps[:].rearrange("p (t n) -> p t n", n=32)
                mv = mx0[:, c * 256 : (c + 1) * 256].rearrange("p (t n) -> p t n", n=16)
                nc.scalar.activation(mv, pv[:, :, 0:16], ACTF.Identity)
                nc.vector.tensor_tensor(out=mv, in0=mv, in1=pv[:, :, 16:32], op=ALU.max)
            # L4 fold into mx0 (chunks of <=512)
            nl4 = max(1, GT * 16 // 512)
            w4 = GT * 16 // nl4
            for c in range(nl4):
                ps = proj_chunk([(OFF[4] + c * w4, OFF[4] + (c + 1) * w4)])
                nc.vector.tensor_tensor(
                    out=mx0[:, c * w4 : (c + 1) * w4],
                    in0=mx0[:, c * w4 : (c + 1) * w4],
                    in1=ps[:], op=ALU.max,
                )
            halve(mx1[:, 0 : GT * 8], mx0[:, 0 : GT * 16].rearrange("p (t n) -> p t n", n=16), 16)
            # L3
            ps = proj_chunk([(OFF[3], OFF[4])])
            nc.vector.tensor_tensor(
                out=mx1[:, 0 : GT * 8], in0=mx1[:, 0 : GT * 8], in1=ps[:], op=ALU.max
            )
            halve(mx2[:, 0 : GT * 4], mx1[:, 0 : GT * 8].rearrange("p (t n) -> p t n", n=8), 8)
            # L0+L1+L2 (contiguous) + L6: one chunk
            ps = proj_chunk([(OFF[0], OFF[3]), (OFF[6], OFF[7])])
            nc.vector.tensor_tensor(
                out=mx2[:, 0 : GT * 4], in0=mx2[:, 0 : GT * 4],
                in1=ps[:, OFF[2] : OFF[3]], op=ALU.max,
            )
            halve(mx3[:, 0 : GT * 2], mx2[:, 0 : GT * 4].rearrange("p (t n) -> p t n", n=4), 4)
            nc.vector.tensor_tensor(
                out=mx3[:, 0 : GT * 2], in0=mx3[:, 0 : GT * 2],
                in1=ps[:, OFF[1] : OFF[2]], op=ALU.max,
            )
            halve(mx4[:, 0:GT], mx3[:, 0 : GT * 2].rearrange("p (t n) -> p t n", n=2), 2)
            nc.vector.tensor_tensor(
                out=mx4[:, 0:GT], in0=mx4[:, 0:GT], in1=ps[:, OFF[0] : OFF[1]],
                op=ALU.max,
            )
            nc.vector.tensor_tensor(
                out=mx4[:, 0:GT], in0=mx4[:, 0:GT],
                in1=ps[:, OFF[3] : OFF[3] + GT], op=ALU.max,
            )

            # relu + scatter into seq2 (fwd slot k=stmt, bwd slot k=31-stmt)
            cg = PAIRC[g]
            lanes = slice(0, 16)
            in_lo = mx4[:, 0 : cg * 16].rearrange("p (si r) -> p si r", r=16)
            in_hi = mx4[:, cg * 16 : 2 * cg * 16].rearrange("p (si r) -> p si r", r=16)
            lo0, hi0 = SG[g], S - SG[g] - cg

            def relu_to(out_view, in_view):
                veng.tensor_scalar(
                    out=out_view, in0=in_view, scalar1=0.0, scalar2=None,
                    op0=ALU.max,
                )

            relu_to(seq2_r[:, lo0 : lo0 + cg, 0, lanes], in_lo)
            relu_to(seq2_r[:, hi0 : hi0 + cg, 0, lanes], in_hi)
            relu_to(seq2_r[:, hi0 : hi0 + cg, 1, lanes][:, ::-1, :], in_lo)
            relu_to(seq2_r[:, lo0 : lo0 + cg, 1, lanes][:, ::-1, :], in_hi)

        # ---------------- Gi matmuls for one group ------------------------
        def gi_group(g: int):
            cg = PAIRC[g]
            w = cg * 16
            for k0 in (SG[g], S - SG[g] - cg):
                gp = gipool.tile([128, 384], FP32, tag="gi")
                for di, d in enumerate(("f", "b")):
                    for gate in range(3):
                        c0 = gate * 128 + di * 64
                        nc.tensor.matmul(
                            gp[:, c0 : c0 + w],
                            wih[d][:, gate * 128 : (gate + 1) * 128],
                            seq2_r[:, k0 : k0 + cg, di, :],
                            start=True, stop=False,
                        )
                        nc.tensor.matmul(
                            gp[:, c0 : c0 + w],
                            bgi[d][0:1, gate * 128 : (gate + 1) * 128],
                            ones[:, 0:w],
                            start=False, stop=True,
                        )
                # evac r/zn -> girz (one op per dir), n -> gin
                gp_v = gp[:, 0:256].rearrange(
                    "p (gt d x) -> p gt d x", gt=2, d=2
                )[:, :, :, 0:w].rearrange("p gt d (k j) -> p k gt d j", j=16)
                gz_v = girz_r[:, k0 : k0 + cg, 0:64].rearrange(
                    "p k (gt dj) -> p k gt dj", dj=32
                )
                for di in range(2):
                    nc.scalar.activation(
                        gz_v[:, :, :, di * 16 : di * 16 + 16],
                        gp_v[:, :, :, di, :],
                        ACTF.Identity,
                    )
                for di in range(2):
                    nc.scalar.activation(
                        gin_r[:, k0 : k0 + cg, di * 16 : di * 16 + 16],
                        gp[:, 256 + di * 64 : 256 + di * 64 + w].rearrange(
                            "p (k j) -> p k j", j=16
                        ),
                        ACTF.Identity,
                    )

        # ---------------- one fused GRU step ------------------------------
        def gru_step(k: int):
            late = k >= 8
            veng = nc.gpsimd if late else nc.vector
            ps = sppool.tile([128, 96], FP32, tag="step")
            nc.tensor.matmul(
                ps[:], ident[:], girz[:, k * 96 : (k + 1) * 96],
                start=True, stop=False, skip_group_check=True,
            )
            for gate in range(3):  # r, -z, n
                for di, d in enumerate(("f", "b")):
                    c0 = gate * 32 + di * 16
                    nc.tensor.matmul(
                        ps[:, c0 : c0 + 16],
                        whh[d][:, gate * 128 : (gate + 1) * 128],
                        h_all[:, di * 16 : (di + 1) * 16],
                        start=False, stop=True, skip_group_check=True,
                    )
            rz = work.tile([128, 64], FP32, tag="rz")
            nc.scalar.activation(rz[:], ps[:, 0:64], ACTF.Sigmoid)  # [r, 1-z]
            nmul = work.tile([128, 32], FP16, tag="nmul")
            nc.vector.tensor_tensor(
                out=nmul[:], in0=ps[:, 64:96], in1=rz[:, 0:32], op=ALU.mult
            )
            ninp = work.tile([128, 32], FP16, tag="ninp")
            nc.vector.tensor_tensor(
                out=ninp[:], in0=nmul[:], in1=gin[:, k * 32 : (k + 1) * 32], op=ALU.add
            )
            n_t = work.tile([128, 32], FP16, tag="nt")
            nc.scalar.activation(n_t[:], ninp[:], ACTF.Tanh)
            # h' = h + (1-z) * (n - h)
            t1 = work.tile([128, 32], FP16, tag="t1")
            veng.tensor_tensor(out=t1[:], in0=n_t[:], in1=h_all[:], op=ALU.subtract)
            t2 = work.tile([128, 32], FP16, tag="t2")
            veng.tensor_tensor(out=t2[:], in0=t1[:], in1=rz[:, 32:64], op=ALU.mult)
            veng.tensor_tensor(out=h_all[:], in0=t2[:], in1=h_all[:], op=ALU.add)

        # ---------------- emission ----------------------------------------
        for g in range(3):
            gather_group(g)
        for g in range(NGRP):
            tree_group(g)
            gi_group(g)
            for k in range(SG[g], SG[g] + PAIRC[g]):
                gru_step(k)
            if 2 <= g < NGRP - 1:
                gather_group(g + 1)
        for k in range(SG[-1] + PAIRC[-1], S):
            gru_step(k)

        # ---- head: sigmoid(|l - r| @ w_out.T + b_out) --------------------
        hs = work.tile([128, 16], FP32, tag="hs")
        nc.vector.tensor_tensor(
            out=hs[:], in0=h_all[:, 0:16], in1=h_all[:, 16:32], op=ALU.add
        )
        d0 = work.tile([128, 8], FP32, tag="d0")
        nc.vector.tensor_tensor(
            out=d0[:], in0=hs[:, 0:8], in1=hs[:, 8:16], op=ALU.subtract
        )
        dabs = work.tile([128, 8], FP16, tag="dabs")
        nc.scalar.activation(dabs[:], d0[:], ACTF.Abs)
        po = ppool.tile([1, 8], FP32, tag="proj")
        nc.tensor.matmul(po[:], wout[:], dabs[:], start=True, stop=True)
        osb = work.tile([1, 8], FP32, tag="osb")
        nc.scalar.activation(osb[:], po[:], ACTF.Sigmoid, bias=bout[:])
        nc.sync.dma_start(out=out_ext.rearrange("(o j) -> o j", o=1), in_=osb[:])

    nc.compile()
    return nc


_NC_CACHE = None


def _get_nc():
    global _NC_CACHE
    if _NC_CACHE is None:
        _NC_CACHE = build_nc()
    return _NC_CACHE


def make_in_maps(inputs: dict) -> list:
    """Host-side prep: shard + permute tokens, convert/transpose weights."""
    tokens = [
        np.asarray(inputs["tokens1"]).astype(np.int64),
        np.asarray(inputs["tokens2"]).astype(np.int64),
    ]
    w_c = np.asarray(inputs["w_c"], np.float64)
    b_c = np.asarray(inputs["b_c"], np.float64)
    # fold the projection bias into the table: W_c @ (emb + t) = W_c @ emb + b_c
    t_bias = np.linalg.solve(w_c, b_c)
    emb16 = (np.asarray(inputs["emb"], np.float64) + t_bias[None, :]).astype(np.float16)
    b_hh = {d: np.asarray(inputs[f"b_hh_{d}"], np.float32) for d in ("f", "b")}
    b_ih = {d: np.asarray(inputs[f"b_ih_{d}"], np.float32) for d in ("f", "b")}

    ghcst = np.zeros((E, S * 32), np.float16)
    ghv = ghcst.reshape(E, S, 32)
    ghv[:, :, 0:16] = b_hh["f"][256:384][:, None, None]
    ghv[:, :, 16:32] = b_hh["b"][256:384][:, None, None]

    def stack3(wt):  # [384, 128] -> [128, 384] blocks [r, -z, n] transposed
        t = wt.T  # [128, 384]
        return np.concatenate([t[:, 0:128], -t[:, 128:256], t[:, 256:384]], axis=1)

    def bgi_rows(d):  # [1, 384]: bsum_r, -bsum_z, b_ih_n
        br = b_ih[d][0:128] + b_hh[d][0:128]
        bz = b_ih[d][128:256] + b_hh[d][128:256]
        return np.concatenate([br, -bz, b_ih[d][256:384]])[None, :].astype(np.float16)

    rep = {
        "emb16": emb16,
        "wcT": np.ascontiguousarray(w_c.T).astype(np.float16),
        "wih": np.stack(
            [np.ascontiguousarray(stack3(np.asarray(inputs[f"w_ih_{d}"], np.float32))).astype(np.float16) for d in ("f", "b")]
        ),
        "whh": np.stack(
            [np.ascontiguousarray(stack3(np.asarray(inputs[f"w_hh_{d}"], np.float32))).astype(np.float16) for d in ("f", "b")]
        ),
        "bgi": np.stack([bgi_rows(d) for d in ("f", "b")]),
        "ghcst": ghcst,
        "wout": np.ascontiguousarray(np.asarray(inputs["w_out"], np.float32).T).astype(np.float16),
        "bout": np.asarray(inputs["b_out"], np.float32).reshape(1, 1),
    }
    in_maps = []
    for i in range(NCORES):
        m = dict(rep)
        both = np.concatenate(
            [tokens[0][i * NL : (i + 1) * NL], tokens[1][i * NL : (i + 1) * NL]]
        )
        for g in range(NGRP):
            m[f"idx{g}"] = _wrap_idx(both[_PERMS[g]])
        in_maps.append(m)
    return in_maps


def kernel(**inputs) -> np.ndarray:
    nc = _get_nc()
    in_maps = make_in_maps(inputs)
    res = run_bass_kernel_spmd(nc, in_maps, list(range(NCORES)))
    out = np.concatenate(
        [np.asarray(res.results[i]["out"], np.float32).reshape(BL, 1) for i in range(NCORES)],
        axis=0,
    )
    return out


# revision 41
# speedup vs baseline: 2.6012x; 1.0459x over previous
"""Trainium2 Bass kernel for nn_BatchProgramCC (tree-GRU program classifier).

Contract: kernel(**inputs) takes FULL unsharded inputs (numpy), returns the
FULL [64, 1] float32 output. Internally shards the B=64 program batch over
8 NeuronCores (8 programs / core), runs one SPMD Bass program, gathers.

Hardcoded problem shape (from the reference):
  V=30000 vocab, E=ENC=H=128, B=64, S=32 statements, K=64 nodes/tree,
  heap tree layout (parent of j is (j-1)//2 within each 64-node block),
  LEVELS=8 level-sync iterations == exact subtree sums (depth 6).

Pipeline per core (per side, 16384 nodes):
  - transpose-mode dma_gather straight from the fp16 embedding table in
    DRAM -> gT[E, node] in SBUF, node columns in level-major order,
    split into 5 two-ended statement groups (pair counts 2,4,4,4,2) so
    the fused fwd+bwd GRU can start early and finish soon after the
    last gather.
  - level-synchronous subtree sums in-place on gT (fp16; pair-sums on
    Pool, accumulate on DVE).
  - projection W_c @ S per <=512-col chunk on PE (fp16), + b_c x size
    rank-1 matmul into the same PSUM; max-folding reads PSUM once
    (Act seeds + DVE folds; GPSIMD cannot touch PSUM on HW).
  - relu + scatter into per-(step, dir) GRU input layout (Pool).
  - fused fwd+bwd GRU, 4 gates (r, z, 1-z, n) so the blend is
    h' = (1-z)*n + z*h with two short tensor ops after tanh.
"""

import numpy as np
from contextlib import ExitStack

try:
    import concourse.bass as bass
except ImportError:  # pragma: no cover
    import sys

    sys.path.insert(0, "/opt/trn_rl_repo")
    import concourse.bass as bass

import concourse.bacc as bacc
import concourse.mybir as mybir
import concourse.tile as tile
from concourse import masks
from concourse.bass_utils import run_bass_kernel_spmd

FP32 = mybir.dt.float32
FP16 = mybir.dt.float16
I16 = mybir.dt.int16
ALU = mybir.AluOpType
ACTF = mybir.ActivationFunctionType

V = 30000
E = 128          # embedding/encode/hidden dim (all 128)
NCORES = 8
BL = 8           # programs per core
S = 32           # statements (GRU steps)
K = 64           # nodes per tree
NL = BL * S * K  # nodes per side per core = 16384

PAIRC = [2, 4, 4, 4, 2]          # statement pairs per two-ended group
NGRP = len(PAIRC)
SG = [0] * NGRP                  # first step of each group
for _g in range(1, NGRP):
    SG[_g] = SG[_g - 1] + PAIRC[_g - 1]
GTREES = [2 * c * BL for c in PAIRC]   # trees per (group, side)
GNODE = [t * K for t in GTREES]        # nodes per (group, side)
GBASE = [0] * NGRP
for _g in range(1, NGRP):
    GBASE[_g] = GBASE[_g - 1] + GNODE[_g - 1]
assert GBASE[-1] + GNODE[-1] == NL

# ---------------------------------------------------------------------------
# Tree level layout (level-major, recursive even/odd split ordering).
# sigma[l] = heap positions of level l, ordered so that the children of
# sigma[l][i] are sigma[l+1][i] (left) and sigma[l+1][i + n_l] (right).
# ---------------------------------------------------------------------------
_SIGMA = [[0]]
for _l in range(1, 6):
    _prev = _SIGMA[-1]
    _SIGMA.append([2 * p + 1 for p in _prev] + [2 * p + 2 for p in _prev])
_SIGMA.append([63])  # level 6: only heap node 63 (= left child of 31)
_NLVL = [len(s) for s in _SIGMA]            # [1, 2, 4, 8, 16, 32, 1]


def _offsets(gt: int) -> list:
    off = [0]
    for n in _NLVL:
        off.append(off[-1] + gt * n)
    return off


_OFFS = [_offsets(2 * t) for t in GTREES]  # sides merged: 2x trees per group

# subtree sizes per heap position
_SZS = [0] * K
for _j in range(K - 1, -1, -1):
    _SZS[_j] = 1
    if 2 * _j + 1 < K:
        _SZS[_j] += _SZS[2 * _j + 1]
    if 2 * _j + 2 < K:
        _SZS[_j] += _SZS[2 * _j + 2]


def _group_stmts(g: int) -> list:
    lo = [SG[g] + i for i in range(PAIRC[g])]
    hi0 = S - SG[g] - PAIRC[g]
    hi = [hi0 + i for i in range(PAIRC[g])]
    return lo + hi


def _group_perm(g: int) -> np.ndarray:
    """perm[col] = (side, within-core node index) for level-major col of
    group g; sides interleaved at the lane level (t_local = si*16+side*8+p).
    Encoded as side * NL + node."""
    stmts = _group_stmts(g)
    order = []
    for lvl in range(7):
        for t_local in range(2 * GTREES[g]):
            si, side, p = t_local // 16, (t_local // 8) % 2, t_local % 8
            tree_global = p * S + stmts[si]
            for hp in _SIGMA[lvl]:
                order.append(side * NL + tree_global * K + hp)
    return np.asarray(order, dtype=np.int64)


_PERMS = [_group_perm(g) for g in range(NGRP)]


def _szs_levelmajor(gt: int) -> np.ndarray:
    return np.asarray(
        [_SZS[hp] for lvl in range(7) for _t in range(gt) for hp in _SIGMA[lvl]],
        dtype=np.float16,
    )


def _wrap_idx(tokens_perm: np.ndarray) -> np.ndarray:
    """Wrap permuted token list into dma_gather idx layout
    [128, n//16] int16 (idx j -> row j%16, col j//16, replicated x8)."""
    n = len(tokens_perm)
    out = np.zeros((128, n // 16), dtype=np.int16)
    blk = tokens_perm.reshape(n // 16, 16).T.astype(np.int16)
    for grp in range(8):
        out[grp * 16 : (grp + 1) * 16, :] = blk
    return out


# ---------------------------------------------------------------------------
# Kernel builder
# ---------------------------------------------------------------------------

def build_nc() -> bass.Bass:
    nc = bacc.Bacc("TRN2", target_bir_lowering=False)

    emb16 = nc.declare_dram_parameter("emb16", [V, E], FP16, isOutput=False)
    idx_ext = [
        nc.declare_dram_parameter(f"idx{g}", [128, 2 * GNODE[g] // 16], I16, isOutput=False)
        for g in range(NGRP)
    ]
    wcT_ext = nc.declare_dram_parameter("wcT", [E, E], FP16, isOutput=False)
    wih_ext = nc.declare_dram_parameter("wih", [2, E, 3 * E], FP16, isOutput=False)
    whh_ext = nc.declare_dram_parameter("whh", [2, E, 3 * E], FP16, isOutput=False)
    bgi_ext = nc.declare_dram_parameter("bgi", [2, 1, 3 * E], FP16, isOutput=False)
    ghcst_ext = nc.declare_dram_parameter("ghcst", [E, S * 32], FP16, isOutput=False)
    wout_ext = nc.declare_dram_parameter("wout", [E, 1], FP16, isOutput=False)
    bout_ext = nc.declare_dram_parameter("bout", [1, 1], FP32, isOutput=False)
    out_ext = nc.declare_dram_parameter("out", [BL], FP32, isOutput=True)

    with tile.TileContext(nc) as tc, ExitStack() as ctx:
        persist = ctx.enter_context(tc.tile_pool(name="persist", bufs=1))
        work = ctx.enter_context(tc.tile_pool(name="work", bufs=2))

        # ---- prologue: constants + weights -----------------------------
        ident = persist.tile([128, 128], FP16)
        masks.make_identity(nc, ident[:])
        ones = persist.tile([1, 64], FP16)
        nc.vector.memset(ones[:], 1.0)
        # touch every activation function once so table switches happen now
        actwarm = persist.tile([1, 4], FP32)
        nc.scalar.activation(actwarm[:], ones[0:1, 0:4], ACTF.Sigmoid)
        nc.scalar.activation(actwarm[:], ones[0:1, 0:4], ACTF.Tanh)
        nc.scalar.activation(actwarm[:], ones[0:1, 0:4], ACTF.Abs)
        nc.scalar.activation(actwarm[:], ones[0:1, 0:4], ACTF.Identity)

        idx_sb = []
        for i in range(NGRP):
            it = persist.tile([128, 2 * GNODE[i] // 16], I16, name=f"idx{i}")
            nc.sync.dma_start(out=it[:], in_=idx_ext[i][:, :])
            idx_sb.append(it)

        wcT = persist.tile([E, E], FP16)
        nc.sync.dma_start(out=wcT[:], in_=wcT_ext[:, :])
        wih = {}
        whh = {}
        bgi = {}
        for di, d in enumerate(("f", "b")):
            wih[d] = persist.tile([E, 3 * E], FP16, name=f"wih_{d}")
            nc.sync.dma_start(out=wih[d][:], in_=wih_ext[di])
            whh[d] = persist.tile([E, 3 * E], FP16, name=f"whh_{d}")
            nc.sync.dma_start(out=whh[d][:], in_=whh_ext[di])
            bgi[d] = persist.tile([1, 3 * E], FP16, name=f"bgi_{d}")
            nc.sync.dma_start(out=bgi[d][:], in_=bgi_ext[di])
        wout = persist.tile([E, 1], FP16)
        nc.sync.dma_start(out=wout[:], in_=wout_ext[:, :])
        bout = persist.tile([1, 1], FP32)
        nc.sync.dma_start(out=bout[:], in_=bout_ext[:, :])

        # GRU state + precomputed-input buffers
        # girz step block (96 cols): [rf rb znf znb ghnf ghnb]
        girz = persist.tile([128, S * 96], FP16)
        girz_r = girz[:].rearrange("p (k c) -> p k c", c=96)
        nc.sync.dma_start(
            out=girz_r[:, :, 64:96],
            in_=ghcst_ext.rearrange("p (k c) -> p k c", c=32),
        )
        gin = persist.tile([128, S * 32], FP16)   # step block: nf(16) nb(16)
        gin_r = gin[:].rearrange("p (k c) -> p k c", c=32)
        seq2 = persist.tile([128, S * 32], FP16)  # col = k*32 + d*16 + side*8 + prog
        seq2_r = seq2[:].rearrange("p (k d l) -> p k d l", d=2, l=16)
        h_all = persist.tile([128, 32], FP16)     # [hf(16) hb(16)], lane = side*8+prog
        nc.vector.memset(h_all[:], 0.0)

        gT = persist.tile([128, 2 * NL], FP16)

        ppool = ctx.enter_context(tc.tile_pool(name="proj", bufs=3, space="PSUM"))
        gipool = ctx.enter_context(tc.tile_pool(name="gip", bufs=2, space="PSUM"))
        sppool = ctx.enter_context(tc.tile_pool(name="stepp", bufs=2, space="PSUM"))


        # ---------------- tree phase for one (group, side) ----------------
        def gather_group(g: int):
            n = 2 * GNODE[g]
            reg = gT[:, 2 * GBASE[g] : 2 * GBASE[g] + n]
            gv = reg.rearrange("p (o n) -> p o n", o=1)
            nc.gpsimd.dma_gather(
                gv, emb16[:, :], idx_sb[g][:], n, n, E,
                transpose=True, single_packet=False,
            )

        def sums_group(g: int):
            OFF = _OFFS[g]
            GT = 2 * GTREES[g]
            reg = gT[:, 2 * GBASE[g] : 2 * GBASE[g] + 2 * GNODE[g]]

            # subtree sums, level-synchronous, in place (fp16, SBUF only)
            seng = nc.gpsimd if g >= 2 else nc.vector
            l5 = reg[:, OFF[5] : OFF[6]].rearrange("p (t n) -> p t n", n=32)
            seng.tensor_tensor(
                out=l5[:, :, 0:1], in0=l5[:, :, 0:1],
                in1=reg[:, OFF[6] : OFF[7]].rearrange("p (t n) -> p t n", n=1),
                op=ALU.add,
            )
            tmp = work.tile([128, 2 * GTREES[1] * 16], FP16, tag="tmp")
            for lvl in range(4, -1, -1):
                n_l = _NLVL[lvl]
                fd = GT * n_l
                child = reg[:, OFF[lvl + 1] : OFF[lvl + 2]].rearrange(
                    "p (t n) -> p t n", n=2 * n_l
                )
                tv = tmp[:, 0:fd].rearrange("p (t n) -> p t n", n=n_l)
                seng.tensor_tensor(
                    out=tv, in0=child[:, :, 0:n_l], in1=child[:, :, n_l : 2 * n_l],
                    op=ALU.add,
                )
                seng.tensor_tensor(
                    out=reg[:, OFF[lvl] : OFF[lvl] + fd],
                    in0=reg[:, OFF[lvl] : OFF[lvl] + fd],
                    in1=tmp[:, 0:fd],
                    op=ALU.add,
                )

        def folds_group(g: int):
            OFF = _OFFS[g]
            GT = 2 * GTREES[g]
            reg = gT[:, 2 * GBASE[g] : 2 * GBASE[g] + 2 * GNODE[g]]
            veng = nc.gpsimd if g >= 2 else nc.vector

            # projection chunks + bias + max folds (PSUM read once:
            # Act seeds the L5 left halves, DVE does all PSUM max-folds)
            def proj_chunk(ranges, mmax=512):
                width = sum(b - a for a, b in ranges)
                ps = ppool.tile([128, width], FP32, tag="proj")
                c0 = 0
                for a, b in ranges:
                    while a < b:
                        w = min(b - a, mmax)
                        nc.tensor.matmul(
                            ps[:, c0 : c0 + w], wcT[:], reg[:, a : a + w],
                            start=True, stop=True,
                        )
                        a += w
                        c0 += w
                return ps

            mx0 = work.tile([128, 2 * GTREES[1] * 16], FP16, tag="mx0")
            mx1 = work.tile([128, 2 * GTREES[1] * 8], FP16, tag="mx1")
            mx2 = work.tile([128, 2 * GTREES[1] * 4], FP16, tag="mx2")
            mx3 = work.tile([128, 2 * GTREES[1] * 2], FP16, tag="mx3")
            mx4 = work.tile([128, 2 * GTREES[1]], FP16, tag="mx4")

            veng = nc.gpsimd if g >= 2 else nc.vector

            def halve(dst, src, n):
                nc.vector.tensor_tensor(
                    out=dst.rearrange("p (t n) -> p t n", n=n // 2),
                    in0=src[:, :, 0 : n // 2], in1=src[:, :, n // 2 : n],
                    op=ALU.max,
                )

            # L5 -> mx0 (chunks of <=512)
            nl5 = GT * 32 // 512
            for c in range(nl5):
                ps = proj_chunk([(OFF[5] + c * 512, OFF[5] + (c + 1) * 512)])
                pv = ps[:].rearrange("p (t n) -> p t n", n=32)
                mv = mx0[:, c * 256 : (c + 1) * 256].rearrange("p (t n) -> p t n", n=16)
                nc.scalar.activation(mv, pv[:, :, 0:16], ACTF.Identity)
                nc.vector.tensor_tensor(out=mv, in0=mv, in1=pv[:, :, 16:32], op=ALU.max)
            # L4 fold into mx0 (chunks of <=512); Act evacuates to SBUF so
            # the DVE fold runs in fp16 2x mode
            s4 = work.tile([128, 2 * GTREES[1] * 16], FP16, tag="s4")
            nl4 = max(1, GT * 16 // 512)
            w4 = GT * 16 // nl4
            for c in range(nl4):
                ps = proj_chunk([(OFF[4] + c * w4, OFF[4] + (c + 1) * w4)])
                nc.scalar.activation(
                    s4[:, c * w4 : (c + 1) * w4], ps[:], ACTF.Identity
                )
                nc.vector.tensor_tensor(
                    out=mx0[:, c * w4 : (c + 1) * w4],
                    in0=mx0[:, c * w4 : (c + 1) * w4],
                    in1=s4[:, c * w4 : (c + 1) * w4], op=ALU.max,
                )
            halve(mx1[:, 0 : GT * 8], mx0[:, 0 : GT * 16].rearrange("p (t n) -> p t n", n=16), 16)
            # L3 (chunks of <=512)
            nl3 = max(1, GT * 8 // 512)
            w3 = GT * 8 // nl3
            for c in range(nl3):
                ps = proj_chunk([(OFF[3] + c * w3, OFF[3] + (c + 1) * w3)])
                nc.vector.tensor_tensor(
                    out=mx1[:, c * w3 : (c + 1) * w3],
                    in0=mx1[:, c * w3 : (c + 1) * w3], in1=ps[:], op=ALU.max,
                )
            halve(mx2[:, 0 : GT * 4], mx1[:, 0 : GT * 8].rearrange("p (t n) -> p t n", n=8), 8)
            if GT * 4 >= 512:
                # big merged group: L2 alone, then L0+L1+L6 packed
                ps2 = proj_chunk([(OFF[2], OFF[3])])
                nc.vector.tensor_tensor(
                    out=mx2[:, 0 : GT * 4], in0=mx2[:, 0 : GT * 4], in1=ps2[:],
                    op=ALU.max,
                )
                ps = proj_chunk([(OFF[0], OFF[2]), (OFF[6], OFF[7])])
                l1o, l6o = GT, 3 * GT
            else:
                ps = proj_chunk([(OFF[0], OFF[3]), (OFF[6], OFF[7])])
                nc.vector.tensor_tensor(
                    out=mx2[:, 0 : GT * 4], in0=mx2[:, 0 : GT * 4],
                    in1=ps[:, OFF[2] : OFF[3]], op=ALU.max,
                )
                l1o, l6o = OFF[1], OFF[3]
            halve(mx3[:, 0 : GT * 2], mx2[:, 0 : GT * 4].rearrange("p (t n) -> p t n", n=4), 4)
            nc.vector.tensor_tensor(
                out=mx3[:, 0 : GT * 2], in0=mx3[:, 0 : GT * 2],
                in1=ps[:, l1o : l1o + GT * 2], op=ALU.max,
            )
            halve(mx4[:, 0:GT], mx3[:, 0 : GT * 2].rearrange("p (t n) -> p t n", n=2), 2)
            nc.vector.tensor_tensor(
                out=mx4[:, 0:GT], in0=mx4[:, 0:GT], in1=ps[:, 0:GT],
                op=ALU.max,
            )
            nc.vector.tensor_tensor(
                out=mx4[:, 0:GT], in0=mx4[:, 0:GT],
                in1=ps[:, l6o : l6o + GT], op=ALU.max,
            )

            # relu + scatter into seq2 (fwd slot k=stmt, bwd slot k=31-stmt)
            cg = PAIRC[g]
            lanes = slice(0, 16)
            in_lo = mx4[:, 0 : cg * 16].rearrange("p (si r) -> p si r", r=16)
            in_hi = mx4[:, cg * 16 : 2 * cg * 16].rearrange("p (si r) -> p si r", r=16)
            lo0, hi0 = SG[g], S - SG[g] - cg

            def relu_to(out_view, in_view):
                veng.tensor_scalar(
                    out=out_view, in0=in_view, scalar1=0.0, scalar2=None,
                    op0=ALU.max,
                )

            relu_to(seq2_r[:, lo0 : lo0 + cg, 0, lanes], in_lo)
            relu_to(seq2_r[:, hi0 : hi0 + cg, 0, lanes], in_hi)
            relu_to(seq2_r[:, hi0 : hi0 + cg, 1, lanes][:, ::-1, :], in_lo)
            relu_to(seq2_r[:, lo0 : lo0 + cg, 1, lanes][:, ::-1, :], in_hi)

        # ---------------- Gi matmuls for one group ------------------------
        def gi_group(g: int):
            cg = PAIRC[g]
            w = cg * 16
            for k0 in (SG[g], S - SG[g] - cg):
                gp = gipool.tile([128, 384], FP32, tag="gi")
                for di, d in enumerate(("f", "b")):
                    for gate in range(3):
                        c0 = gate * 128 + di * 64
                        nc.tensor.matmul(
                            gp[:, c0 : c0 + w],
                            wih[d][:, gate * 128 : (gate + 1) * 128],
                            seq2_r[:, k0 : k0 + cg, di, :],
                            start=True, stop=False,
                        )
                        nc.tensor.matmul(
                            gp[:, c0 : c0 + w],
                            bgi[d][0:1, gate * 128 : (gate + 1) * 128],
                            ones[:, 0:w],
                            start=False, stop=True,
                        )
                # evac r/zn -> girz (one op per dir), n -> gin
                gp_v = gp[:, 0:256].rearrange(
                    "p (gt d x) -> p gt d x", gt=2, d=2
                )[:, :, :, 0:w].rearrange("p gt d (k j) -> p k gt d j", j=16)
                gz_v = girz_r[:, k0 : k0 + cg, 0:64].rearrange(
                    "p k (gt dj) -> p k gt dj", dj=32
                )
                for di in range(2):
                    nc.scalar.activation(
                        gz_v[:, :, :, di * 16 : di * 16 + 16],
                        gp_v[:, :, :, di, :],
                        ACTF.Identity,
                    )
                for di in range(2):
                    nc.scalar.activation(
                        gin_r[:, k0 : k0 + cg, di * 16 : di * 16 + 16],
                        gp[:, 256 + di * 64 : 256 + di * 64 + w].rearrange(
                            "p (k j) -> p k j", j=16
                        ),
                        ACTF.Identity,
                    )

        # ---------------- one fused GRU step ------------------------------
        def gru_step(k: int):
            late = k >= 8
            veng = nc.gpsimd if late else nc.vector
            ps = sppool.tile([128, 96], FP32, tag="step")
            nc.tensor.matmul(
                ps[:], ident[:], girz[:, k * 96 : (k + 1) * 96],
                start=True, stop=False, skip_group_check=True,
            )
            for gate in range(3):  # r, -z, n
                for di, d in enumerate(("f", "b")):
                    c0 = gate * 32 + di * 16
                    nc.tensor.matmul(
                        ps[:, c0 : c0 + 16],
                        whh[d][:, gate * 128 : (gate + 1) * 128],
                        h_all[:, di * 16 : (di + 1) * 16],
                        start=False, stop=True, skip_group_check=True,
                    )
            rz = work.tile([128, 64], FP32, tag="rz")
            nc.scalar.activation(rz[:], ps[:, 0:64], ACTF.Sigmoid)  # [r, 1-z]
            nmul = work.tile([128, 32], FP16, tag="nmul")
            nc.vector.tensor_tensor(
                out=nmul[:], in0=ps[:, 64:96], in1=rz[:, 0:32], op=ALU.mult
            )
            ninp = work.tile([128, 32], FP16, tag="ninp")
            nc.vector.tensor_tensor(
                out=ninp[:], in0=nmul[:], in1=gin[:, k * 32 : (k + 1) * 32], op=ALU.add
            )
            n_t = work.tile([128, 32], FP16, tag="nt")
            nc.scalar.activation(n_t[:], ninp[:], ACTF.Tanh)
            # h' = h + (1-z) * (n - h)
            t1 = work.tile([128, 32], FP16, tag="t1")
            veng.tensor_tensor(out=t1[:], in0=n_t[:], in1=h_all[:], op=ALU.subtract)
            t2 = work.tile([128, 32], FP16, tag="t2")
            veng.tensor_tensor(out=t2[:], in0=t1[:], in1=rz[:, 32:64], op=ALU.mult)
            veng.tensor_tensor(out=h_all[:], in0=t2[:], in1=h_all[:], op=ALU.add)

        # ---------------- emission ----------------------------------------
        for g in range(3):
            gather_group(g)
        for g in range(NGRP):
            sums_group(g)
            folds_group(g)
            gi_group(g)
            for k in range(SG[g], SG[g] + PAIRC[g]):
                gru_step(k)
            if 2 <= g < NGRP - 1:
                gather_group(g + 1)
        for k in range(SG[-1] + PAIRC[-1], S):
            gru_step(k)

        # ---- head: sigmoid(|l - r| @ w_out.T + b_out) --------------------
        hs = work.tile([128, 16], FP32, tag="hs")
        nc.vector.tensor_tensor(
            out=hs[:], in0=h_all[:, 0:16], in1=h_all[:, 16:32], op=ALU.add
        )
        d0 = work.tile([128, 8], FP32, tag="d0")
        nc.vector.tensor_tensor(
            out=d0[:], in0=hs[:, 0:8], in1=hs[:, 8:16], op=ALU.subtract
        )
        dabs = work.tile([128, 8], FP16, tag="dabs")
        nc.scalar.activation(dabs[:], d0[:], ACTF.Abs)
        po = ppool.tile([1, 8], FP32, tag="proj")
        nc.tensor.matmul(po[:], wout[:], dabs[:], start=True, stop=True)
        osb = work.tile([1, 8], FP32, tag="osb")
        nc.scalar.activation(osb[:], po[:], ACTF.Sigmoid, bias=bout[:])
        nc.sync.dma_start(out=out_ext.rearrange("(o j) -> o j", o=1), in_=osb[:])

    nc.compile()
    return nc


_NC_CACHE = None


def _get_nc():
    global _NC_CACHE
    if _NC_CACHE is None:
        _NC_CACHE = build_nc()
    return _NC_CACHE


def make_in_maps(inputs: dict) -> list:
    """Host-side prep: shard + permute tokens, convert/transpose weights."""
    tokens = [
        np.asarray(inputs["tokens1"]).astype(np.int64),
        np.asarray(inputs["tokens2"]).astype(np.int64),
    ]
    w_c = np.asarray(inputs["w_c"], np.float64)
    b_c = np.asarray(inputs["b_c"], np.float64)
    # fold the projection bias into the table: W_c @ (emb + t) = W_c @ emb + b_c
    t_bias = np.linalg.solve(w_c, b_c)
    emb16 = (np.asarray(inputs["emb"], np.float64) + t_bias[None, :]).astype(np.float16)
    b_hh = {d: np.asarray(inputs[f"b_hh_{d}"], np.float32) for d in ("f", "b")}
    b_ih = {d: np.asarray(inputs[f"b_ih_{d}"], np.float32) for d in ("f", "b")}

    ghcst = np.zeros((E, S * 32), np.float16)
    ghv = ghcst.reshape(E, S, 32)
    ghv[:, :, 0:16] = b_hh["f"][256:384][:, None, None]
    ghv[:, :, 16:32] = b_hh["b"][256:384][:, None, None]

    def stack3(wt):  # [384, 128] -> [128, 384] blocks [r, -z, n] transposed
        t = wt.T  # [128, 384]
        return np.concatenate([t[:, 0:128], -t[:, 128:256], t[:, 256:384]], axis=1)

    def bgi_rows(d):  # [1, 384]: bsum_r, -bsum_z, b_ih_n
        br = b_ih[d][0:128] + b_hh[d][0:128]
        bz = b_ih[d][128:256] + b_hh[d][128:256]
        return np.concatenate([br, -bz, b_ih[d][256:384]])[None, :].astype(np.float16)

    rep = {
        "emb16": emb16,
        "wcT": np.ascontiguousarray(w_c.T).astype(np.float16),
        "wih": np.stack(
            [np.ascontiguousarray(stack3(np.asarray(inputs[f"w_ih_{d}"], np.float32))).astype(np.float16) for d in ("f", "b")]
        ),
        "whh": np.stack(
            [np.ascontiguousarray(stack3(np.asarray(inputs[f"w_hh_{d}"], np.float32))).astype(np.float16) for d in ("f", "b")]
        ),
        "bgi": np.stack([bgi_rows(d) for d in ("f", "b")]),
        "ghcst": ghcst,
        "wout": np.ascontiguousarray(np.asarray(inputs["w_out"], np.float32).T).astype(np.float16),
        "bout": np.asarray(inputs["b_out"], np.float32).reshape(1, 1),
    }
    in_maps = []
    for i in range(NCORES):
        m = dict(rep)
        both = np.concatenate(
            [tokens[0][i * NL : (i + 1) * NL], tokens[1][i * NL : (i + 1) * NL]]
        )
        for g in range(NGRP):
            m[f"idx{g}"] = _wrap_idx(both[_PERMS[g]])
        in_maps.append(m)
    return in_maps


def kernel(**inputs) -> np.ndarray:
    nc = _get_nc()
    in_maps = make_in_maps(inputs)
    res = run_bass_kernel_spmd(nc, in_maps, list(range(NCORES)))
    out = np.concatenate(
        [np.asarray(res.results[i]["out"], np.float32).reshape(BL, 1) for i in range(NCORES)],
        axis=0,
    )
    return out


# revision 52
# speedup vs baseline: 2.6634x; 1.0239x over previous
"""Trainium2 Bass kernel for nn_BatchProgramCC (tree-GRU program classifier).

Contract: kernel(**inputs) takes FULL unsharded inputs (numpy), returns the
FULL [64, 1] float32 output. Internally shards the B=64 program batch over
8 NeuronCores (8 programs / core), runs one SPMD Bass program, gathers.

Hardcoded problem shape (from the reference):
  V=30000 vocab, E=ENC=H=128, B=64, S=32 statements, K=64 nodes/tree,
  heap tree layout (parent of j is (j-1)//2 within each 64-node block),
  LEVELS=8 level-sync iterations == exact subtree sums (depth 6).

Pipeline per core (per side, 16384 nodes):
  - transpose-mode dma_gather straight from the fp16 embedding table in
    DRAM -> gT[E, node] in SBUF, node columns in level-major order,
    split into 5 two-ended statement groups (pair counts 2,4,4,4,2) so
    the fused fwd+bwd GRU can start early and finish soon after the
    last gather.
  - level-synchronous subtree sums in-place on gT (fp16; pair-sums on
    Pool, accumulate on DVE).
  - projection W_c @ S per <=512-col chunk on PE (fp16), + b_c x size
    rank-1 matmul into the same PSUM; max-folding reads PSUM once
    (Act seeds + DVE folds; GPSIMD cannot touch PSUM on HW).
  - relu + scatter into per-(step, dir) GRU input layout (Pool).
  - fused fwd+bwd GRU, 4 gates (r, z, 1-z, n) so the blend is
    h' = (1-z)*n + z*h with two short tensor ops after tanh.
"""

import numpy as np
from contextlib import ExitStack

try:
    import concourse.bass as bass
except ImportError:  # pragma: no cover
    import sys

    sys.path.insert(0, "/opt/trn_rl_repo")
    import concourse.bass as bass

import concourse.bacc as bacc
import concourse.mybir as mybir
import concourse.tile as tile
from concourse import masks
from concourse.bass_utils import run_bass_kernel_spmd

FP32 = mybir.dt.float32
FP16 = mybir.dt.float16
I16 = mybir.dt.int16
ALU = mybir.AluOpType
ACTF = mybir.ActivationFunctionType

V = 30000
E = 128          # embedding/encode/hidden dim (all 128)
NCORES = 8
BL = 8           # programs per core
S = 32           # statements (GRU steps)
K = 64           # nodes per tree
NL = BL * S * K  # nodes per side per core = 16384

PAIRC = [2, 4, 4, 4, 2]          # statement pairs per two-ended group
NGRP = len(PAIRC)
SG = [0] * NGRP                  # first step of each group
for _g in range(1, NGRP):
    SG[_g] = SG[_g - 1] + PAIRC[_g - 1]
GTREES = [2 * c * BL for c in PAIRC]   # trees per (group, side)
GNODE = [t * K for t in GTREES]        # nodes per (group, side)
GBASE = [0] * NGRP
for _g in range(1, NGRP):
    GBASE[_g] = GBASE[_g - 1] + GNODE[_g - 1]
assert GBASE[-1] + GNODE[-1] == NL

# ---------------------------------------------------------------------------
# Tree level layout (level-major, recursive even/odd split ordering).
# sigma[l] = heap positions of level l, ordered so that the children of
# sigma[l][i] are sigma[l+1][i] (left) and sigma[l+1][i + n_l] (right).
# ---------------------------------------------------------------------------
_SIGMA = [[0]]
for _l in range(1, 6):
    _prev = _SIGMA[-1]
    _SIGMA.append([2 * p + 1 for p in _prev] + [2 * p + 2 for p in _prev])
_SIGMA.append([63])  # level 6: only heap node 63 (= left child of 31)
_NLVL = [len(s) for s in _SIGMA]            # [1, 2, 4, 8, 16, 32, 1]


def _offsets(gt: int) -> list:
    off = [0]
    for n in _NLVL:
        off.append(off[-1] + gt * n)
    return off


_OFFS = [_offsets(2 * t) for t in GTREES]  # sides merged: 2x trees per group

# subtree sizes per heap position
_SZS = [0] * K
for _j in range(K - 1, -1, -1):
    _SZS[_j] = 1
    if 2 * _j + 1 < K:
        _SZS[_j] += _SZS[2 * _j + 1]
    if 2 * _j + 2 < K:
        _SZS[_j] += _SZS[2 * _j + 2]


def _group_stmts(g: int) -> list:
    lo = [SG[g] + i for i in range(PAIRC[g])]
    hi0 = S - SG[g] - PAIRC[g]
    hi = [hi0 + i for i in range(PAIRC[g])]
    return lo + hi


def _group_perm(g: int) -> np.ndarray:
    """perm[col] = (side, within-core node index) for level-major col of
    group g; sides interleaved at the lane level (t_local = si*16+side*8+p).
    Encoded as side * NL + node."""
    stmts = _group_stmts(g)
    order = []
    for lvl in range(7):
        for t_local in range(2 * GTREES[g]):
            si, side, p = t_local // 16, (t_local // 8) % 2, t_local % 8
            tree_global = p * S + stmts[si]
            for hp in _SIGMA[lvl]:
                order.append(side * NL + tree_global * K + hp)
    return np.asarray(order, dtype=np.int64)


_PERMS = [_group_perm(g) for g in range(NGRP)]


def _szs_levelmajor(gt: int) -> np.ndarray:
    return np.asarray(
        [_SZS[hp] for lvl in range(7) for _t in range(gt) for hp in _SIGMA[lvl]],
        dtype=np.float16,
    )


def _wrap_idx(tokens_perm: np.ndarray) -> np.ndarray:
    """Wrap permuted token list into dma_gather idx layout
    [128, n//16] int16 (idx j -> row j%16, col j//16, replicated x8)."""
    n = len(tokens_perm)
    out = np.zeros((128, n // 16), dtype=np.int16)
    blk = tokens_perm.reshape(n // 16, 16).T.astype(np.int16)
    for grp in range(8):
        out[grp * 16 : (grp + 1) * 16, :] = blk
    return out


# ---------------------------------------------------------------------------
# Kernel builder
# ---------------------------------------------------------------------------

def build_nc() -> bass.Bass:
    nc = bacc.Bacc("TRN2", target_bir_lowering=False)

    emb16 = nc.declare_dram_parameter("emb16", [V, E], FP16, isOutput=False)
    idx_ext = [
        nc.declare_dram_parameter(f"idx{g}", [128, 2 * GNODE[g] // 16], I16, isOutput=False)
        for g in range(NGRP)
    ]
    wcT_ext = nc.declare_dram_parameter("wcT", [E, E], FP16, isOutput=False)
    wih_ext = nc.declare_dram_parameter("wih", [2, E, 3 * E], FP16, isOutput=False)
    whh_ext = nc.declare_dram_parameter("whh", [2, E, 3 * E], FP16, isOutput=False)
    bgi_ext = nc.declare_dram_parameter("bgi", [2, 1, 3 * E], FP16, isOutput=False)
    ghcst_ext = nc.declare_dram_parameter("ghcst", [E, S * 32], FP16, isOutput=False)
    wout_ext = nc.declare_dram_parameter("wout", [E, 1], FP16, isOutput=False)
    bout_ext = nc.declare_dram_parameter("bout", [1, 1], FP32, isOutput=False)
    out_ext = nc.declare_dram_parameter("out", [BL], FP32, isOutput=True)

    with tile.TileContext(nc) as tc, ExitStack() as ctx:
        persist = ctx.enter_context(tc.tile_pool(name="persist", bufs=1))
        work = ctx.enter_context(tc.tile_pool(name="work", bufs=2))

        # ---- prologue: constants + weights -----------------------------
        ident = persist.tile([128, 128], FP16)
        masks.make_identity(nc, ident[:])
        ones = persist.tile([1, 64], FP16)
        nc.vector.memset(ones[:], 1.0)
        # touch every activation function once so table switches happen now
        actwarm = persist.tile([1, 4], FP32)
        nc.scalar.activation(actwarm[:], ones[0:1, 0:4], ACTF.Sigmoid)
        nc.scalar.activation(actwarm[:], ones[0:1, 0:4], ACTF.Tanh)
        nc.scalar.activation(actwarm[:], ones[0:1, 0:4], ACTF.Abs)
        nc.scalar.activation(actwarm[:], ones[0:1, 0:4], ACTF.Identity)

        idx_sb = []
        for i in range(NGRP):
            it = persist.tile([128, 2 * GNODE[i] // 16], I16, name=f"idx{i}")
            nc.sync.dma_start(out=it[:], in_=idx_ext[i][:, :])
            idx_sb.append(it)

        wcT = persist.tile([E, E], FP16)
        nc.sync.dma_start(out=wcT[:], in_=wcT_ext[:, :])
        wih = {}
        whh = {}
        bgi = {}
        for di, d in enumerate(("f", "b")):
            wih[d] = persist.tile([E, 3 * E], FP16, name=f"wih_{d}")
            nc.sync.dma_start(out=wih[d][:], in_=wih_ext[di])
            whh[d] = persist.tile([E, 3 * E], FP16, name=f"whh_{d}")
            nc.sync.dma_start(out=whh[d][:], in_=whh_ext[di])
            bgi[d] = persist.tile([1, 3 * E], FP16, name=f"bgi_{d}")
            nc.sync.dma_start(out=bgi[d][:], in_=bgi_ext[di])
        wout = persist.tile([E, 1], FP16)
        nc.sync.dma_start(out=wout[:], in_=wout_ext[:, :])
        bout = persist.tile([1, 1], FP32)
        nc.sync.dma_start(out=bout[:], in_=bout_ext[:, :])

        # GRU state + precomputed-input buffers
        # girz step block (96 cols): [rf rb znf znb ghnf ghnb]
        girz = persist.tile([128, S * 96], FP16)
        girz_r = girz[:].rearrange("p (k c) -> p k c", c=96)
        nc.sync.dma_start(
            out=girz_r[:, :, 64:96],
            in_=ghcst_ext.rearrange("p (k c) -> p k c", c=32),
        )
        gin = persist.tile([128, S * 32], FP16)   # step block: nf(16) nb(16)
        gin_r = gin[:].rearrange("p (k c) -> p k c", c=32)
        seq2 = persist.tile([128, S * 32], FP16)  # col = k*32 + d*16 + side*8 + prog
        seq2_r = seq2[:].rearrange("p (k d l) -> p k d l", d=2, l=16)
        h_all = persist.tile([128, 32], FP16)     # [hf(16) hb(16)], lane = side*8+prog
        nc.vector.memset(h_all[:], 0.0)

        gT = persist.tile([128, 2 * NL], FP16)

        ppool = ctx.enter_context(tc.tile_pool(name="proj", bufs=3, space="PSUM"))
        gipool = ctx.enter_context(tc.tile_pool(name="gip", bufs=2, space="PSUM"))
        sppool = ctx.enter_context(tc.tile_pool(name="stepp", bufs=2, space="PSUM"))


        # ---------------- tree phase for one (group, side) ----------------
        def gather_group(g: int):
            n = 2 * GNODE[g]
            reg = gT[:, 2 * GBASE[g] : 2 * GBASE[g] + n]
            gv = reg.rearrange("p (o n) -> p o n", o=1)
            nc.gpsimd.dma_gather(
                gv, emb16[:, :], idx_sb[g][:], n, n, E,
                transpose=True, single_packet=False,
            )

        def sums_group(g: int):
            OFF = _OFFS[g]
            GT = 2 * GTREES[g]
            reg = gT[:, 2 * GBASE[g] : 2 * GBASE[g] + 2 * GNODE[g]]

            # subtree sums, level-synchronous, in place (fp16, SBUF only)
            seng = nc.gpsimd if g >= 2 else nc.vector
            l5 = reg[:, OFF[5] : OFF[6]].rearrange("p (t n) -> p t n", n=32)
            seng.tensor_tensor(
                out=l5[:, :, 0:1], in0=l5[:, :, 0:1],
                in1=reg[:, OFF[6] : OFF[7]].rearrange("p (t n) -> p t n", n=1),
                op=ALU.add,
            )
            tmp = work.tile([128, 2 * GTREES[1] * 16], FP16, tag="tmp")
            for lvl in range(4, -1, -1):
                n_l = _NLVL[lvl]
                fd = GT * n_l
                child = reg[:, OFF[lvl + 1] : OFF[lvl + 2]].rearrange(
                    "p (t n) -> p t n", n=2 * n_l
                )
                tv = tmp[:, 0:fd].rearrange("p (t n) -> p t n", n=n_l)
                seng.tensor_tensor(
                    out=tv, in0=child[:, :, 0:n_l], in1=child[:, :, n_l : 2 * n_l],
                    op=ALU.add,
                )
                seng.tensor_tensor(
                    out=reg[:, OFF[lvl] : OFF[lvl] + fd],
                    in0=reg[:, OFF[lvl] : OFF[lvl] + fd],
                    in1=tmp[:, 0:fd],
                    op=ALU.add,
                )

        def folds_group(g: int):
            OFF = _OFFS[g]
            GT = 2 * GTREES[g]
            reg = gT[:, 2 * GBASE[g] : 2 * GBASE[g] + 2 * GNODE[g]]
            veng = nc.gpsimd if g >= 1 else nc.vector

            # projection chunks + bias + max folds (PSUM read once:
            # Act seeds the L5 left halves, DVE does all PSUM max-folds)
            def proj_chunk(ranges, mmax=512):
                width = sum(b - a for a, b in ranges)
                ps = ppool.tile([128, width], FP32, tag="proj")
                c0 = 0
                for a, b in ranges:
                    while a < b:
                        w = min(b - a, mmax)
                        nc.tensor.matmul(
                            ps[:, c0 : c0 + w], wcT[:], reg[:, a : a + w],
                            start=True, stop=True,
                        )
                        a += w
                        c0 += w
                return ps

            mx0 = work.tile([128, 2 * GTREES[1] * 16], FP16, tag="mx0")
            mx1 = work.tile([128, 2 * GTREES[1] * 8], FP16, tag="mx1")
            mx2 = work.tile([128, 2 * GTREES[1] * 4], FP16, tag="mx2")
            mx3 = work.tile([128, 2 * GTREES[1] * 2], FP16, tag="mx3")
            mx4 = work.tile([128, 2 * GTREES[1]], FP16, tag="mx4")

            veng = nc.gpsimd if g >= 1 else nc.vector

            def halve(dst, src, n):
                nc.vector.tensor_tensor(
                    out=dst.rearrange("p (t n) -> p t n", n=n // 2),
                    in0=src[:, :, 0 : n // 2], in1=src[:, :, n // 2 : n],
                    op=ALU.max,
                )

            # L5 -> mx0 (chunks of <=512)
            nl5 = GT * 32 // 512
            for c in range(nl5):
                ps = proj_chunk([(OFF[5] + c * 512, OFF[5] + (c + 1) * 512)])
                pv = ps[:].rearrange("p (t n) -> p t n", n=32)
                mv = mx0[:, c * 256 : (c + 1) * 256].rearrange("p (t n) -> p t n", n=16)
                nc.scalar.activation(mv, pv[:, :, 0:16], ACTF.Identity)
                nc.vector.tensor_tensor(out=mv, in0=mv, in1=pv[:, :, 16:32], op=ALU.max)
            # L4 fold into mx0 (chunks of <=512)
            nl4 = max(1, GT * 16 // 512)
            w4 = GT * 16 // nl4
            for c in range(nl4):
                ps = proj_chunk([(OFF[4] + c * w4, OFF[4] + (c + 1) * w4)])
                nc.vector.tensor_tensor(
                    out=mx0[:, c * w4 : (c + 1) * w4],
                    in0=mx0[:, c * w4 : (c + 1) * w4],
                    in1=ps[:], op=ALU.max,
                )
            halve(mx1[:, 0 : GT * 8], mx0[:, 0 : GT * 16].rearrange("p (t n) -> p t n", n=16), 16)
            # L3 (chunks of <=512)
            nl3 = max(1, GT * 8 // 512)
            w3 = GT * 8 // nl3
            for c in range(nl3):
                ps = proj_chunk([(OFF[3] + c * w3, OFF[3] + (c + 1) * w3)])
                nc.vector.tensor_tensor(
                    out=mx1[:, c * w3 : (c + 1) * w3],
                    in0=mx1[:, c * w3 : (c + 1) * w3], in1=ps[:], op=ALU.max,
                )
            halve(mx2[:, 0 : GT * 4], mx1[:, 0 : GT * 8].rearrange("p (t n) -> p t n", n=8), 8)
            if GT * 4 >= 512:
                # big merged group: L2 alone, then L0+L1+L6 packed
                ps2 = proj_chunk([(OFF[2], OFF[3])])
                nc.vector.tensor_tensor(
                    out=mx2[:, 0 : GT * 4], in0=mx2[:, 0 : GT * 4], in1=ps2[:],
                    op=ALU.max,
                )
                ps = proj_chunk([(OFF[0], OFF[2]), (OFF[6], OFF[7])])
                l1o, l6o = GT, 3 * GT
            else:
                ps = proj_chunk([(OFF[0], OFF[3]), (OFF[6], OFF[7])])
                nc.vector.tensor_tensor(
                    out=mx2[:, 0 : GT * 4], in0=mx2[:, 0 : GT * 4],
                    in1=ps[:, OFF[2] : OFF[3]], op=ALU.max,
                )
                l1o, l6o = OFF[1], OFF[3]
            halve(mx3[:, 0 : GT * 2], mx2[:, 0 : GT * 4].rearrange("p (t n) -> p t n", n=4), 4)
            nc.vector.tensor_tensor(
                out=mx3[:, 0 : GT * 2], in0=mx3[:, 0 : GT * 2],
                in1=ps[:, l1o : l1o + GT * 2], op=ALU.max,
            )
            halve(mx4[:, 0:GT], mx3[:, 0 : GT * 2].rearrange("p (t n) -> p t n", n=2), 2)
            nc.vector.tensor_tensor(
                out=mx4[:, 0:GT], in0=mx4[:, 0:GT], in1=ps[:, 0:GT],
                op=ALU.max,
            )
            nc.vector.tensor_tensor(
                out=mx4[:, 0:GT], in0=mx4[:, 0:GT],
                in1=ps[:, l6o : l6o + GT], op=ALU.max,
            )

            # relu + scatter into seq2 (fwd slot k=stmt, bwd slot k=31-stmt)
            cg = PAIRC[g]
            lanes = slice(0, 16)
            in_lo = mx4[:, 0 : cg * 16].rearrange("p (si r) -> p si r", r=16)
            in_hi = mx4[:, cg * 16 : 2 * cg * 16].rearrange("p (si r) -> p si r", r=16)
            lo0, hi0 = SG[g], S - SG[g] - cg

            def relu_to(out_view, in_view):
                veng.tensor_scalar(
                    out=out_view, in0=in_view, scalar1=0.0, scalar2=None,
                    op0=ALU.max,
                )

            relu_to(seq2_r[:, lo0 : lo0 + cg, 0, lanes], in_lo)
            relu_to(seq2_r[:, hi0 : hi0 + cg, 0, lanes], in_hi)
            relu_to(seq2_r[:, hi0 : hi0 + cg, 1, lanes][:, ::-1, :], in_lo)
            relu_to(seq2_r[:, lo0 : lo0 + cg, 1, lanes][:, ::-1, :], in_hi)

        # ---------------- Gi matmuls for one group ------------------------
        def gi_group(g: int):
            cg = PAIRC[g]
            w = cg * 16
            for k0 in (SG[g], S - SG[g] - cg):
                gp = gipool.tile([128, 384], FP32, tag="gi")
                for di, d in enumerate(("f", "b")):
                    for gate in range(3):
                        c0 = gate * 128 + di * 64
                        nc.tensor.matmul(
                            gp[:, c0 : c0 + w],
                            wih[d][:, gate * 128 : (gate + 1) * 128],
                            seq2_r[:, k0 : k0 + cg, di, :],
                            start=True, stop=False,
                        )
                        nc.tensor.matmul(
                            gp[:, c0 : c0 + w],
                            bgi[d][0:1, gate * 128 : (gate + 1) * 128],
                            ones[:, 0:w],
                            start=False, stop=True,
                        )
                # evac r/zn -> girz (one op per dir), n -> gin
                gp_v = gp[:, 0:256].rearrange(
                    "p (gt d x) -> p gt d x", gt=2, d=2
                )[:, :, :, 0:w].rearrange("p gt d (k j) -> p k gt d j", j=16)
                gz_v = girz_r[:, k0 : k0 + cg, 0:64].rearrange(
                    "p k (gt dj) -> p k gt dj", dj=32
                )
                for di in range(2):
                    nc.scalar.activation(
                        gz_v[:, :, :, di * 16 : di * 16 + 16],
                        gp_v[:, :, :, di, :],
                        ACTF.Identity,
                    )
                for di in range(2):
                    nc.scalar.activation(
                        gin_r[:, k0 : k0 + cg, di * 16 : di * 16 + 16],
                        gp[:, 256 + di * 64 : 256 + di * 64 + w].rearrange(
                            "p (k j) -> p k j", j=16
                        ),
                        ACTF.Identity,
                    )

        # ---------------- one fused GRU step ------------------------------
        def gru_step(k: int):
            late = k >= 2
            veng = nc.gpsimd if late else nc.vector
            ps = sppool.tile([128, 96], FP32, tag="step")
            nc.tensor.matmul(
                ps[:], ident[:], girz[:, k * 96 : (k + 1) * 96],
                start=True, stop=False, skip_group_check=True,
            )
            for gate in range(3):  # r, -z, n
                for di, d in enumerate(("f", "b")):
                    c0 = gate * 32 + di * 16
                    nc.tensor.matmul(
                        ps[:, c0 : c0 + 16],
                        whh[d][:, gate * 128 : (gate + 1) * 128],
                        h_all[:, di * 16 : (di + 1) * 16],
                        start=False, stop=True, skip_group_check=True,
                    )
            rz = work.tile([128, 64], FP32, tag="rz")
            nc.scalar.activation(rz[:], ps[:, 0:64], ACTF.Sigmoid)  # [r, 1-z]
            nmul = work.tile([128, 32], FP16, tag="nmul")
            nc.vector.tensor_tensor(
                out=nmul[:], in0=ps[:, 64:96], in1=rz[:, 0:32], op=ALU.mult
            )
            ninp = work.tile([128, 32], FP16, tag="ninp")
            nc.vector.tensor_tensor(
                out=ninp[:], in0=nmul[:], in1=gin[:, k * 32 : (k + 1) * 32], op=ALU.add
            )
            n_t = work.tile([128, 32], FP16, tag="nt")
            nc.scalar.activation(n_t[:], ninp[:], ACTF.Tanh)
            # h' = h + (1-z) * (n - h)
            t1 = work.tile([128, 32], FP16, tag="t1")
            veng.tensor_tensor(out=t1[:], in0=n_t[:], in1=h_all[:], op=ALU.subtract)
            t2 = work.tile([128, 32], FP16, tag="t2")
            veng.tensor_tensor(out=t2[:], in0=t1[:], in1=rz[:, 32:64], op=ALU.mult)
            veng.tensor_tensor(out=h_all[:], in0=t2[:], in1=h_all[:], op=ALU.add)

        # ---------------- emission ----------------------------------------
        for g in range(3):
            gather_group(g)
        for g in range(NGRP):
            sums_group(g)
            folds_group(g)
            gi_group(g)
            for k in range(SG[g], SG[g] + PAIRC[g]):
                gru_step(k)
            if 2 <= g < NGRP - 1:
                gather_group(g + 1)
        for k in range(SG[-1] + PAIRC[-1], S):
            gru_step(k)

        # ---- head: sigmoid(|l - r| @ w_out.T + b_out) --------------------
        hs = work.tile([128, 16], FP32, tag="hs")
        nc.vector.tensor_tensor(
            out=hs[:], in0=h_all[:, 0:16], in1=h_all[:, 16:32], op=ALU.add
        )
        d0 = work.tile([128, 8], FP32, tag="d0")
        nc.vector.tensor_tensor(
            out=d0[:], in0=hs[:, 0:8], in1=hs[:, 8:16], op=ALU.subtract
        )
        dabs = work.tile([128, 8], FP16, tag="dabs")
        nc.scalar.activation(dabs[:], d0[:], ACTF.Abs)
        po = ppool.tile([1, 8], FP32, tag="proj")
        nc.tensor.matmul(po[:], wout[:], dabs[:], start=True, stop=True)
        osb = work.tile([1, 8], FP32, tag="osb")
        nc.scalar.activation(osb[:], po[:], ACTF.Sigmoid, bias=bout[:])
        nc.sync.dma_start(out=out_ext.rearrange("(o j) -> o j", o=1), in_=osb[:])

    nc.compile()
    return nc


_NC_CACHE = None


def _get_nc():
    global _NC_CACHE
    if _NC_CACHE is None:
        _NC_CACHE = build_nc()
    return _NC_CACHE


def make_in_maps(inputs: dict) -> list:
    """Host-side prep: shard + permute tokens, convert/transpose weights."""
    tokens = [
        np.asarray(inputs["tokens1"]).astype(np.int64),
        np.asarray(inputs["tokens2"]).astype(np.int64),
    ]
    w_c = np.asarray(inputs["w_c"], np.float64)
    b_c = np.asarray(inputs["b_c"], np.float64)
    # fold the projection bias into the table: W_c @ (emb + t) = W_c @ emb + b_c
    try:
        t_bias = np.linalg.solve(w_c, b_c)
    except np.linalg.LinAlgError:
        t_bias = np.linalg.lstsq(w_c, b_c, rcond=None)[0]
    emb16 = (np.asarray(inputs["emb"], np.float64) + t_bias[None, :]).astype(np.float16)
    b_hh = {d: np.asarray(inputs[f"b_hh_{d}"], np.float32) for d in ("f", "b")}
    b_ih = {d: np.asarray(inputs[f"b_ih_{d}"], np.float32) for d in ("f", "b")}

    ghcst = np.zeros((E, S * 32), np.float16)
    ghv = ghcst.reshape(E, S, 32)
    ghv[:, :, 0:16] = b_hh["f"][256:384][:, None, None]
    ghv[:, :, 16:32] = b_hh["b"][256:384][:, None, None]

    def stack3(wt):  # [384, 128] -> [128, 384] blocks [r, -z, n] transposed
        t = wt.T  # [128, 384]
        return np.concatenate([t[:, 0:128], -t[:, 128:256], t[:, 256:384]], axis=1)

    def bgi_rows(d):  # [1, 384]: bsum_r, -bsum_z, b_ih_n
        br = b_ih[d][0:128] + b_hh[d][0:128]
        bz = b_ih[d][128:256] + b_hh[d][128:256]
        return np.concatenate([br, -bz, b_ih[d][256:384]])[None, :].astype(np.float16)

    rep = {
        "emb16": emb16,
        "wcT": np.ascontiguousarray(w_c.T).astype(np.float16),
        "wih": np.stack(
            [np.ascontiguousarray(stack3(np.asarray(inputs[f"w_ih_{d}"], np.float32))).astype(np.float16) for d in ("f", "b")]
        ),
        "whh": np.stack(
            [np.ascontiguousarray(stack3(np.asarray(inputs[f"w_hh_{d}"], np.float32))).astype(np.float16) for d in ("f", "b")]
        ),
        "bgi": np.stack([bgi_rows(d) for d in ("f", "b")]),
        "ghcst": ghcst,
        "wout": np.ascontiguousarray(np.asarray(inputs["w_out"], np.float32).T).astype(np.float16),
        "bout": np.asarray(inputs["b_out"], np.float32).reshape(1, 1),
    }
    in_maps = []
    for i in range(NCORES):
        m = dict(rep)
        both = np.concatenate(
            [tokens[0][i * NL : (i + 1) * NL], tokens[1][i * NL : (i + 1) * NL]]
        )
        for g in range(NGRP):
            m[f"idx{g}"] = _wrap_idx(both[_PERMS[g]])
        in_maps.append(m)
    return in_maps


def kernel(**inputs) -> np.ndarray:
    nc = _get_nc()
    in_maps = make_in_maps(inputs)
    res = run_bass_kernel_spmd(nc, in_maps, list(range(NCORES)))
    out = np.concatenate(
        [np.asarray(res.results[i]["out"], np.float32).reshape(BL, 1) for i in range(NCORES)],
        axis=0,
    )
    return out


# revision 62
# speedup vs baseline: 2.6712x; 1.0029x over previous
"""Trainium2 Bass kernel for nn_BatchProgramCC (tree-GRU program classifier).

Contract: kernel(**inputs) takes FULL unsharded inputs (numpy), returns the
FULL [64, 1] float32 output. Internally shards the B=64 program batch over
8 NeuronCores (8 programs / core), runs one SPMD Bass program, gathers.

Hardcoded problem shape (from the reference):
  V=30000 vocab, E=ENC=H=128, B=64, S=32 statements, K=64 nodes/tree,
  heap tree layout (parent of j is (j-1)//2 within each 64-node block),
  LEVELS=8 level-sync iterations == exact subtree sums (depth 6).

Pipeline per core (per side, 16384 nodes):
  - transpose-mode dma_gather straight from the fp16 embedding table in
    DRAM -> gT[E, node] in SBUF, node columns in level-major order,
    both sides merged, split into 5 two-ended statement groups (pair
    counts 2,4,4,4,2) so
    the fused fwd+bwd GRU can start early and finish soon after the
    last gather.
  - level-synchronous subtree sums in-place on gT (fp16; pair-sums on
    Pool, accumulate on DVE).
  - projection W_c @ S per <=512-col chunk on PE (fp16), + b_c x size
    rank-1 matmul into the same PSUM; max-folding reads PSUM once
    (Act seeds + DVE folds; GPSIMD cannot touch PSUM on HW).
  - relu + scatter into per-(step, dir) GRU input layout (Pool).
  - fused fwd+bwd GRU, 3 gates (r, 1-z, n; z-weights sign-folded on the
    host) so the blend is h' = h + (1-z)*(n - h).
  - chained idle matmuls keep the PE p-state ramped so projection
    matmuls run at full clock.
"""

import numpy as np
from contextlib import ExitStack

try:
    import concourse.bass as bass
except ImportError:  # pragma: no cover
    import sys

    sys.path.insert(0, "/opt/trn_rl_repo")
    import concourse.bass as bass

import concourse.bacc as bacc
import concourse.mybir as mybir
import concourse.tile as tile
from concourse import masks
from concourse.bass_utils import run_bass_kernel_spmd

FP32 = mybir.dt.float32
FP16 = mybir.dt.float16
I16 = mybir.dt.int16
ALU = mybir.AluOpType
ACTF = mybir.ActivationFunctionType

V = 30000
E = 128          # embedding/encode/hidden dim (all 128)
NCORES = 8
BL = 8           # programs per core
S = 32           # statements (GRU steps)
K = 64           # nodes per tree
NL = BL * S * K  # nodes per side per core = 16384

PAIRC = [2, 4, 4, 4, 2]          # statement pairs per two-ended group
NGRP = len(PAIRC)
SG = [0] * NGRP                  # first step of each group
for _g in range(1, NGRP):
    SG[_g] = SG[_g - 1] + PAIRC[_g - 1]
GTREES = [2 * c * BL for c in PAIRC]   # trees per (group, side)
GNODE = [t * K for t in GTREES]        # nodes per (group, side)
GBASE = [0] * NGRP
for _g in range(1, NGRP):
    GBASE[_g] = GBASE[_g - 1] + GNODE[_g - 1]
assert GBASE[-1] + GNODE[-1] == NL

# ---------------------------------------------------------------------------
# Tree level layout (level-major, recursive even/odd split ordering).
# sigma[l] = heap positions of level l, ordered so that the children of
# sigma[l][i] are sigma[l+1][i] (left) and sigma[l+1][i + n_l] (right).
# ---------------------------------------------------------------------------
_SIGMA = [[0]]
for _l in range(1, 6):
    _prev = _SIGMA[-1]
    _SIGMA.append([2 * p + 1 for p in _prev] + [2 * p + 2 for p in _prev])
_SIGMA.append([63])  # level 6: only heap node 63 (= left child of 31)
_NLVL = [len(s) for s in _SIGMA]            # [1, 2, 4, 8, 16, 32, 1]


def _offsets(gt: int) -> list:
    off = [0]
    for n in _NLVL:
        off.append(off[-1] + gt * n)
    return off


_OFFS = [_offsets(2 * t) for t in GTREES]  # sides merged: 2x trees per group

# subtree sizes per heap position
_SZS = [0] * K
for _j in range(K - 1, -1, -1):
    _SZS[_j] = 1
    if 2 * _j + 1 < K:
        _SZS[_j] += _SZS[2 * _j + 1]
    if 2 * _j + 2 < K:
        _SZS[_j] += _SZS[2 * _j + 2]


def _group_stmts(g: int) -> list:
    lo = [SG[g] + i for i in range(PAIRC[g])]
    hi0 = S - SG[g] - PAIRC[g]
    hi = [hi0 + i for i in range(PAIRC[g])]
    return lo + hi


def _group_perm(g: int) -> np.ndarray:
    """perm[col] = (side, within-core node index) for level-major col of
    group g; sides interleaved at the lane level (t_local = si*16+side*8+p).
    Encoded as side * NL + node."""
    stmts = _group_stmts(g)
    order = []
    for lvl in range(7):
        for t_local in range(2 * GTREES[g]):
            si, side, p = t_local // 16, (t_local // 8) % 2, t_local % 8
            tree_global = p * S + stmts[si]
            for hp in _SIGMA[lvl]:
                order.append(side * NL + tree_global * K + hp)
    return np.asarray(order, dtype=np.int64)


_PERMS = [_group_perm(g) for g in range(NGRP)]


def _szs_levelmajor(gt: int) -> np.ndarray:
    return np.asarray(
        [_SZS[hp] for lvl in range(7) for _t in range(gt) for hp in _SIGMA[lvl]],
        dtype=np.float16,
    )


def _wrap_idx(tokens_perm: np.ndarray) -> np.ndarray:
    """Wrap permuted token list into dma_gather idx layout
    [128, n//16] int16 (idx j -> row j%16, col j//16, replicated x8)."""
    n = len(tokens_perm)
    out = np.zeros((128, n // 16), dtype=np.int16)
    blk = tokens_perm.reshape(n // 16, 16).T.astype(np.int16)
    for grp in range(8):
        out[grp * 16 : (grp + 1) * 16, :] = blk
    return out


# ---------------------------------------------------------------------------
# Kernel builder
# ---------------------------------------------------------------------------

def build_nc() -> bass.Bass:
    nc = bacc.Bacc("TRN2", target_bir_lowering=False)

    emb16 = nc.declare_dram_parameter("emb16", [V, E], FP16, isOutput=False)
    idx_ext = [
        nc.declare_dram_parameter(f"idx{g}", [128, 2 * GNODE[g] // 16], I16, isOutput=False)
        for g in range(NGRP)
    ]
    wcT_ext = nc.declare_dram_parameter("wcT", [E, E], FP16, isOutput=False)
    wih_ext = nc.declare_dram_parameter("wih", [2, E, 3 * E], FP16, isOutput=False)
    whh_ext = nc.declare_dram_parameter("whh", [2, E, 3 * E], FP16, isOutput=False)
    bgi_ext = nc.declare_dram_parameter("bgi", [2, 1, 3 * E], FP16, isOutput=False)
    ghcst_ext = nc.declare_dram_parameter("ghcst", [E, S * 32], FP16, isOutput=False)
    wout_ext = nc.declare_dram_parameter("wout", [E, 1], FP16, isOutput=False)
    bout_ext = nc.declare_dram_parameter("bout", [1, 1], FP32, isOutput=False)
    out_ext = nc.declare_dram_parameter("out", [BL], FP32, isOutput=True)

    with tile.TileContext(nc) as tc, ExitStack() as ctx:
        persist = ctx.enter_context(tc.tile_pool(name="persist", bufs=1))
        work = ctx.enter_context(tc.tile_pool(name="work", bufs=2))

        # ---- prologue: constants + weights -----------------------------
        ident = persist.tile([128, 128], FP16)
        masks.make_identity(nc, ident[:])
        ones = persist.tile([1, 64], FP16)
        nc.vector.memset(ones[:], 1.0)
        # touch every activation function once so table switches happen now
        actwarm = persist.tile([1, 4], FP32)
        nc.scalar.activation(actwarm[:], ones[0:1, 0:4], ACTF.Sigmoid)
        nc.scalar.activation(actwarm[:], ones[0:1, 0:4], ACTF.Tanh)
        nc.scalar.activation(actwarm[:], ones[0:1, 0:4], ACTF.Abs)
        nc.scalar.activation(actwarm[:], ones[0:1, 0:4], ACTF.Identity)

        idx_sb = []
        for i in range(NGRP):
            it = persist.tile([128, 2 * GNODE[i] // 16], I16, name=f"idx{i}")
            nc.sync.dma_start(out=it[:], in_=idx_ext[i][:, :])
            idx_sb.append(it)

        wcT = persist.tile([E, E], FP16)
        nc.sync.dma_start(out=wcT[:], in_=wcT_ext[:, :])
        wih = {}
        whh = {}
        bgi = {}
        for di, d in enumerate(("f", "b")):
            wih[d] = persist.tile([E, 3 * E], FP16, name=f"wih_{d}")
            nc.sync.dma_start(out=wih[d][:], in_=wih_ext[di])
            whh[d] = persist.tile([E, 3 * E], FP16, name=f"whh_{d}")
            nc.sync.dma_start(out=whh[d][:], in_=whh_ext[di])
            bgi[d] = persist.tile([1, 3 * E], FP16, name=f"bgi_{d}")
            nc.sync.dma_start(out=bgi[d][:], in_=bgi_ext[di])
        wout = persist.tile([E, 1], FP16)
        nc.sync.dma_start(out=wout[:], in_=wout_ext[:, :])
        bout = persist.tile([1, 1], FP32)
        nc.sync.dma_start(out=bout[:], in_=bout_ext[:, :])

        # GRU state + precomputed-input buffers
        # girz step block (96 cols): [rf rb znf znb ghnf ghnb]
        girz = persist.tile([128, S * 96], FP16)
        girz_r = girz[:].rearrange("p (k c) -> p k c", c=96)
        nc.sync.dma_start(
            out=girz_r[:, :, 64:96],
            in_=ghcst_ext.rearrange("p (k c) -> p k c", c=32),
        )
        gin = persist.tile([128, S * 32], FP16)   # step block: nf(16) nb(16)
        gin_r = gin[:].rearrange("p (k c) -> p k c", c=32)
        seq2 = persist.tile([128, S * 32], FP16)  # col = k*32 + d*16 + side*8 + prog
        seq2_r = seq2[:].rearrange("p (k d l) -> p k d l", d=2, l=16)
        h_all = persist.tile([128, 32], FP16)     # [hf(16) hb(16)], lane = side*8+prog
        nc.vector.memset(h_all[:], 0.0)

        gT = persist.tile([128, 2 * NL], FP16)

        warmpool = ctx.enter_context(tc.tile_pool(name="warm", bufs=1, space="PSUM"))
        warm_t = warmpool.tile([128, 512], FP32)

        def pe_warm(n=1):
            for _ in range(n):
                nc.tensor.matmul(
                    warm_t[:], ident[:], gT[:, 0:512],
                    start=True, stop=True, skip_group_check=True,
                )

        ppool = ctx.enter_context(tc.tile_pool(name="proj", bufs=3, space="PSUM"))
        gipool = ctx.enter_context(tc.tile_pool(name="gip", bufs=2, space="PSUM"))
        sppool = ctx.enter_context(tc.tile_pool(name="stepp", bufs=2, space="PSUM"))


        # ---------------- tree phase for one (group, side) ----------------
        def gather_group(g: int):
            n = 2 * GNODE[g]
            reg = gT[:, 2 * GBASE[g] : 2 * GBASE[g] + n]
            gv = reg.rearrange("p (o n) -> p o n", o=1)
            nc.gpsimd.dma_gather(
                gv, emb16[:, :], idx_sb[g][:], n, n, E,
                transpose=True, single_packet=False,
            )

        def sums_group(g: int):
            OFF = _OFFS[g]
            GT = 2 * GTREES[g]
            reg = gT[:, 2 * GBASE[g] : 2 * GBASE[g] + 2 * GNODE[g]]

            # subtree sums, level-synchronous, in place (fp16, SBUF only)
            seng = nc.gpsimd if g >= 2 else nc.vector
            l5 = reg[:, OFF[5] : OFF[6]].rearrange("p (t n) -> p t n", n=32)
            seng.tensor_tensor(
                out=l5[:, :, 0:1], in0=l5[:, :, 0:1],
                in1=reg[:, OFF[6] : OFF[7]].rearrange("p (t n) -> p t n", n=1),
                op=ALU.add,
            )
            tmp = work.tile([128, 2 * max(GTREES) * 16], FP16, tag="tmp")
            for lvl in range(4, -1, -1):
                n_l = _NLVL[lvl]
                fd = GT * n_l
                child = reg[:, OFF[lvl + 1] : OFF[lvl + 2]].rearrange(
                    "p (t n) -> p t n", n=2 * n_l
                )
                tv = tmp[:, 0:fd].rearrange("p (t n) -> p t n", n=n_l)
                seng.tensor_tensor(
                    out=tv, in0=child[:, :, 0:n_l], in1=child[:, :, n_l : 2 * n_l],
                    op=ALU.add,
                )
                seng.tensor_tensor(
                    out=reg[:, OFF[lvl] : OFF[lvl] + fd],
                    in0=reg[:, OFF[lvl] : OFF[lvl] + fd],
                    in1=tmp[:, 0:fd],
                    op=ALU.add,
                )

        def folds_group(g: int):
            OFF = _OFFS[g]
            GT = 2 * GTREES[g]
            reg = gT[:, 2 * GBASE[g] : 2 * GBASE[g] + 2 * GNODE[g]]
            veng = nc.gpsimd if g >= 1 else nc.vector

            # projection chunks + bias + max folds (PSUM read once:
            # Act seeds the L5 left halves, DVE does all PSUM max-folds)
            def proj_chunk(ranges, mmax=512):
                width = sum(b - a for a, b in ranges)
                ps = ppool.tile([128, width], FP32, tag="proj")
                c0 = 0
                for a, b in ranges:
                    while a < b:
                        w = min(b - a, mmax)
                        nc.tensor.matmul(
                            ps[:, c0 : c0 + w], wcT[:], reg[:, a : a + w],
                            start=True, stop=True,
                        )
                        a += w
                        c0 += w
                return ps

            mx0 = work.tile([128, 2 * max(GTREES) * 16], FP16, tag="mx0")
            mx1 = work.tile([128, 2 * max(GTREES) * 8], FP16, tag="mx1")
            mx2 = work.tile([128, 2 * max(GTREES) * 4], FP16, tag="mx2")
            mx3 = work.tile([128, 2 * max(GTREES) * 2], FP16, tag="mx3")
            mx4 = work.tile([128, 2 * max(GTREES)], FP16, tag="mx4")

            veng = nc.gpsimd if g >= 1 else nc.vector

            def halve(dst, src, n):
                nc.vector.tensor_tensor(
                    out=dst.rearrange("p (t n) -> p t n", n=n // 2),
                    in0=src[:, :, 0 : n // 2], in1=src[:, :, n // 2 : n],
                    op=ALU.max,
                )

            # L5 -> mx0 (chunks of <=512)
            nl5 = GT * 32 // 512
            for c in range(nl5):
                ps = proj_chunk([(OFF[5] + c * 512, OFF[5] + (c + 1) * 512)])
                pv = ps[:].rearrange("p (t n) -> p t n", n=32)
                mv = mx0[:, c * 256 : (c + 1) * 256].rearrange("p (t n) -> p t n", n=16)
                nc.scalar.activation(mv, pv[:, :, 0:16], ACTF.Identity)
                nc.vector.tensor_tensor(out=mv, in0=mv, in1=pv[:, :, 16:32], op=ALU.max)
            # L4 fold into mx0 (chunks of <=512)
            nl4 = max(1, GT * 16 // 512)
            w4 = GT * 16 // nl4
            for c in range(nl4):
                ps = proj_chunk([(OFF[4] + c * w4, OFF[4] + (c + 1) * w4)])
                nc.vector.tensor_tensor(
                    out=mx0[:, c * w4 : (c + 1) * w4],
                    in0=mx0[:, c * w4 : (c + 1) * w4],
                    in1=ps[:], op=ALU.max,
                )
            halve(mx1[:, 0 : GT * 8], mx0[:, 0 : GT * 16].rearrange("p (t n) -> p t n", n=16), 16)
            # L3 (chunks of <=512)
            nl3 = max(1, GT * 8 // 512)
            w3 = GT * 8 // nl3
            for c in range(nl3):
                ps = proj_chunk([(OFF[3] + c * w3, OFF[3] + (c + 1) * w3)])
                nc.vector.tensor_tensor(
                    out=mx1[:, c * w3 : (c + 1) * w3],
                    in0=mx1[:, c * w3 : (c + 1) * w3], in1=ps[:], op=ALU.max,
                )
            halve(mx2[:, 0 : GT * 4], mx1[:, 0 : GT * 8].rearrange("p (t n) -> p t n", n=8), 8)
            if GT * 4 >= 512:
                # big merged group: L2 alone, then L0+L1+L6 packed
                ps2 = proj_chunk([(OFF[2], OFF[3])])
                nc.vector.tensor_tensor(
                    out=mx2[:, 0 : GT * 4], in0=mx2[:, 0 : GT * 4], in1=ps2[:],
                    op=ALU.max,
                )
                ps = proj_chunk([(OFF[0], OFF[2]), (OFF[6], OFF[7])])
                l1o, l6o = GT, 3 * GT
            else:
                ps = proj_chunk([(OFF[0], OFF[3]), (OFF[6], OFF[7])])
                nc.vector.tensor_tensor(
                    out=mx2[:, 0 : GT * 4], in0=mx2[:, 0 : GT * 4],
                    in1=ps[:, OFF[2] : OFF[3]], op=ALU.max,
                )
                l1o, l6o = OFF[1], OFF[3]
            halve(mx3[:, 0 : GT * 2], mx2[:, 0 : GT * 4].rearrange("p (t n) -> p t n", n=4), 4)
            nc.vector.tensor_tensor(
                out=mx3[:, 0 : GT * 2], in0=mx3[:, 0 : GT * 2],
                in1=ps[:, l1o : l1o + GT * 2], op=ALU.max,
            )
            halve(mx4[:, 0:GT], mx3[:, 0 : GT * 2].rearrange("p (t n) -> p t n", n=2), 2)
            nc.vector.tensor_tensor(
                out=mx4[:, 0:GT], in0=mx4[:, 0:GT], in1=ps[:, 0:GT],
                op=ALU.max,
            )
            nc.vector.tensor_tensor(
                out=mx4[:, 0:GT], in0=mx4[:, 0:GT],
                in1=ps[:, l6o : l6o + GT], op=ALU.max,
            )

            # relu + scatter into seq2 (fwd slot k=stmt, bwd slot k=31-stmt)
            cg = PAIRC[g]
            lanes = slice(0, 16)
            in_lo = mx4[:, 0 : cg * 16].rearrange("p (si r) -> p si r", r=16)
            in_hi = mx4[:, cg * 16 : 2 * cg * 16].rearrange("p (si r) -> p si r", r=16)
            lo0, hi0 = SG[g], S - SG[g] - cg

            def relu_to(out_view, in_view):
                veng.tensor_scalar(
                    out=out_view, in0=in_view, scalar1=0.0, scalar2=None,
                    op0=ALU.max,
                )

            relu_to(seq2_r[:, lo0 : lo0 + cg, 0, lanes], in_lo)
            relu_to(seq2_r[:, hi0 : hi0 + cg, 0, lanes], in_hi)
            relu_to(seq2_r[:, hi0 : hi0 + cg, 1, lanes][:, ::-1, :], in_lo)
            relu_to(seq2_r[:, lo0 : lo0 + cg, 1, lanes][:, ::-1, :], in_hi)

        # ---------------- Gi matmuls for one group ------------------------
        def gi_group(g: int):
            cg = PAIRC[g]
            w = cg * 16
            for k0 in (SG[g], S - SG[g] - cg):
                gp = gipool.tile([128, 384], FP32, tag="gi")
                for di, d in enumerate(("f", "b")):
                    for gate in range(3):
                        c0 = gate * 128 + di * 64
                        nc.tensor.matmul(
                            gp[:, c0 : c0 + w],
                            wih[d][:, gate * 128 : (gate + 1) * 128],
                            seq2_r[:, k0 : k0 + cg, di, :],
                            start=True, stop=False,
                        )
                        nc.tensor.matmul(
                            gp[:, c0 : c0 + w],
                            bgi[d][0:1, gate * 128 : (gate + 1) * 128],
                            ones[:, 0:w],
                            start=False, stop=True,
                        )
                # evac r/zn -> girz (one op per dir), n -> gin
                gp_v = gp[:, 0:256].rearrange(
                    "p (gt d x) -> p gt d x", gt=2, d=2
                )[:, :, :, 0:w].rearrange("p gt d (k j) -> p k gt d j", j=16)
                gz_v = girz_r[:, k0 : k0 + cg, 0:64].rearrange(
                    "p k (gt dj) -> p k gt dj", dj=32
                )
                for di in range(2):
                    nc.scalar.activation(
                        gz_v[:, :, :, di * 16 : di * 16 + 16],
                        gp_v[:, :, :, di, :],
                        ACTF.Identity,
                    )
                for di in range(2):
                    nc.scalar.activation(
                        gin_r[:, k0 : k0 + cg, di * 16 : di * 16 + 16],
                        gp[:, 256 + di * 64 : 256 + di * 64 + w].rearrange(
                            "p (k j) -> p k j", j=16
                        ),
                        ACTF.Identity,
                    )

        # ---------------- one fused GRU step ------------------------------
        def gru_step(k: int):
            late = k >= 2
            veng = nc.gpsimd if late else nc.vector
            ps = sppool.tile([128, 96], FP32, tag="step")
            nc.tensor.matmul(
                ps[:], ident[:], girz[:, k * 96 : (k + 1) * 96],
                start=True, stop=False, skip_group_check=True,
            )
            for gate in range(3):  # r, -z, n
                for di, d in enumerate(("f", "b")):
                    c0 = gate * 32 + di * 16
                    nc.tensor.matmul(
                        ps[:, c0 : c0 + 16],
                        whh[d][:, gate * 128 : (gate + 1) * 128],
                        h_all[:, di * 16 : (di + 1) * 16],
                        start=False, stop=True, skip_group_check=True,
                    )
            rz = work.tile([128, 64], FP32, tag="rz")
            nc.scalar.activation(rz[:], ps[:, 0:64], ACTF.Sigmoid)  # [r, 1-z]
            nmul = work.tile([128, 32], FP16, tag="nmul")
            nc.vector.tensor_tensor(
                out=nmul[:], in0=ps[:, 64:96], in1=rz[:, 0:32], op=ALU.mult
            )
            ninp = work.tile([128, 32], FP16, tag="ninp")
            nc.vector.tensor_tensor(
                out=ninp[:], in0=nmul[:], in1=gin[:, k * 32 : (k + 1) * 32], op=ALU.add
            )
            n_t = work.tile([128, 32], FP16, tag="nt")
            nc.scalar.activation(n_t[:], ninp[:], ACTF.Tanh)
            # h' = h + (1-z) * (n - h)
            t1 = work.tile([128, 32], FP16, tag="t1")
            veng.tensor_tensor(out=t1[:], in0=n_t[:], in1=h_all[:], op=ALU.subtract)
            t2 = work.tile([128, 32], FP16, tag="t2")
            veng.tensor_tensor(out=t2[:], in0=t1[:], in1=rz[:, 32:64], op=ALU.mult)
            veng.tensor_tensor(out=h_all[:], in0=t2[:], in1=h_all[:], op=ALU.add)
            pe_warm(2)

        # ---------------- emission ----------------------------------------
        for g in range(3):
            gather_group(g)
        for g in range(NGRP):
            sums_group(g)
            pe_warm(4)
            folds_group(g)
            gi_group(g)
            for k in range(SG[g], SG[g] + PAIRC[g]):
                gru_step(k)
            if 2 <= g < NGRP - 1:
                gather_group(g + 1)
        for k in range(SG[-1] + PAIRC[-1], S):
            gru_step(k)

        # ---- head: sigmoid(|l - r| @ w_out.T + b_out) --------------------
        hs = work.tile([128, 16], FP32, tag="hs")
        nc.vector.tensor_tensor(
            out=hs[:], in0=h_all[:, 0:16], in1=h_all[:, 16:32], op=ALU.add
        )
        d0 = work.tile([128, 8], FP32, tag="d0")
        nc.vector.tensor_tensor(
            out=d0[:], in0=hs[:, 0:8], in1=hs[:, 8:16], op=ALU.subtract
        )
        dabs = work.tile([128, 8], FP16, tag="dabs")
        nc.scalar.activation(dabs[:], d0[:], ACTF.Abs)
        po = ppool.tile([1, 8], FP32, tag="proj")
        nc.tensor.matmul(po[:], wout[:], dabs[:], start=True, stop=True)
        osb = work.tile([1, 8], FP32, tag="osb")
        nc.scalar.activation(osb[:], po[:], ACTF.Sigmoid, bias=bout[:])
        nc.sync.dma_start(out=out_ext.rearrange("(o j) -> o j", o=1), in_=osb[:])

    nc.compile()
    return nc


_NC_CACHE = None


def _get_nc():
    global _NC_CACHE
    if _NC_CACHE is None:
        _NC_CACHE = build_nc()
    return _NC_CACHE


def make_in_maps(inputs: dict) -> list:
    """Host-side prep: shard + permute tokens, convert/transpose weights."""
    tokens = [
        np.asarray(inputs["tokens1"]).astype(np.int64),
        np.asarray(inputs["tokens2"]).astype(np.int64),
    ]
    w_c = np.asarray(inputs["w_c"], np.float64)
    b_c = np.asarray(inputs["b_c"], np.float64)
    # fold the projection bias into the table: W_c @ (emb + t) = W_c @ emb + b_c
    try:
        t_bias = np.linalg.solve(w_c, b_c)
    except np.linalg.LinAlgError:
        t_bias = np.linalg.lstsq(w_c, b_c, rcond=None)[0]
    emb16 = (np.asarray(inputs["emb"], np.float64) + t_bias[None, :]).astype(np.float16)
    b_hh = {d: np.asarray(inputs[f"b_hh_{d}"], np.float32) for d in ("f", "b")}
    b_ih = {d: np.asarray(inputs[f"b_ih_{d}"], np.float32) for d in ("f", "b")}

    ghcst = np.zeros((E, S * 32), np.float16)
    ghv = ghcst.reshape(E, S, 32)
    ghv[:, :, 0:16] = b_hh["f"][256:384][:, None, None]
    ghv[:, :, 16:32] = b_hh["b"][256:384][:, None, None]

    def stack3(wt):  # [384, 128] -> [128, 384] blocks [r, -z, n] transposed
        t = wt.T  # [128, 384]
        return np.concatenate([t[:, 0:128], -t[:, 128:256], t[:, 256:384]], axis=1)

    def bgi_rows(d):  # [1, 384]: bsum_r, -bsum_z, b_ih_n
        br = b_ih[d][0:128] + b_hh[d][0:128]
        bz = b_ih[d][128:256] + b_hh[d][128:256]
        return np.concatenate([br, -bz, b_ih[d][256:384]])[None, :].astype(np.float16)

    rep = {
        "emb16": emb16,
        "wcT": np.ascontiguousarray(w_c.T).astype(np.float16),
        "wih": np.stack(
            [np.ascontiguousarray(stack3(np.asarray(inputs[f"w_ih_{d}"], np.float32))).astype(np.float16) for d in ("f", "b")]
        ),
        "whh": np.stack(
            [np.ascontiguousarray(stack3(np.asarray(inputs[f"w_hh_{d}"], np.float32))).astype(np.float16) for d in ("f", "b")]
        ),
        "bgi": np.stack([bgi_rows(d) for d in ("f", "b")]),
        "ghcst": ghcst,
        "wout": np.ascontiguousarray(np.asarray(inputs["w_out"], np.float32).T).astype(np.float16),
        "bout": np.asarray(inputs["b_out"], np.float32).reshape(1, 1),
    }
    in_maps = []
    for i in range(NCORES):
        m = dict(rep)
        both = np.concatenate(
            [tokens[0][i * NL : (i + 1) * NL], tokens[1][i * NL : (i + 1) * NL]]
        )
        for g in range(NGRP):
            m[f"idx{g}"] = _wrap_idx(both[_PERMS[g]])
        in_maps.append(m)
    return in_maps


def kernel(**inputs) -> np.ndarray:
    nc = _get_nc()
    in_maps = make_in_maps(inputs)
    res = run_bass_kernel_spmd(nc, in_maps, list(range(NCORES)))
    out = np.concatenate(
        [np.asarray(res.results[i]["out"], np.float32).reshape(BL, 1) for i in range(NCORES)],
        axis=0,
    )
    return out


# revision 67
# speedup vs baseline: 2.6947x; 1.0088x over previous
"""Trainium2 Bass kernel for nn_BatchProgramCC (tree-GRU program classifier).

Contract: kernel(**inputs) takes FULL unsharded inputs (numpy), returns the
FULL [64, 1] float32 output. Internally shards the B=64 program batch over
8 NeuronCores (8 programs / core), runs one SPMD Bass program, gathers.

Hardcoded problem shape (from the reference):
  V=30000 vocab, E=ENC=H=128, B=64, S=32 statements, K=64 nodes/tree,
  heap tree layout (parent of j is (j-1)//2 within each 64-node block),
  LEVELS=8 level-sync iterations == exact subtree sums (depth 6).

Pipeline per core (per side, 16384 nodes):
  - transpose-mode dma_gather straight from the fp16 embedding table in
    DRAM -> gT[E, node] in SBUF, node columns in level-major order,
    both sides merged, split into 5 two-ended statement groups (pair
    counts 2,4,4,4,2) so
    the fused fwd+bwd GRU can start early and finish soon after the
    last gather.
  - level-synchronous subtree sums in-place on gT (fp16; pair-sums on
    Pool, accumulate on DVE).
  - projection W_c @ S per <=512-col chunk on PE (fp16), + b_c x size
    rank-1 matmul into the same PSUM; max-folding reads PSUM once
    (Act seeds + DVE folds; GPSIMD cannot touch PSUM on HW).
  - relu + scatter into per-(step, dir) GRU input layout (Pool).
  - fused fwd+bwd GRU, 3 gates (r, 1-z, n; z-weights sign-folded on the
    host) so the blend is h' = h + (1-z)*(n - h).
  - chained idle matmuls keep the PE p-state ramped so projection
    matmuls run at full clock.
"""

import numpy as np
from contextlib import ExitStack

try:
    import concourse.bass as bass
except ImportError:  # pragma: no cover
    import sys

    sys.path.insert(0, "/opt/trn_rl_repo")
    import concourse.bass as bass

import concourse.bacc as bacc
import concourse.mybir as mybir
import concourse.tile as tile
from concourse import masks
from concourse.bass_utils import run_bass_kernel_spmd

FP32 = mybir.dt.float32
FP16 = mybir.dt.float16
I16 = mybir.dt.int16
ALU = mybir.AluOpType
ACTF = mybir.ActivationFunctionType

V = 30000
E = 128          # embedding/encode/hidden dim (all 128)
NCORES = 8
BL = 8           # programs per core
S = 32           # statements (GRU steps)
K = 64           # nodes per tree
NL = BL * S * K  # nodes per side per core = 16384

PAIRC = [2, 4, 4, 4, 2]          # statement pairs per two-ended group
NGRP = len(PAIRC)
SG = [0] * NGRP                  # first step of each group
for _g in range(1, NGRP):
    SG[_g] = SG[_g - 1] + PAIRC[_g - 1]
GTREES = [2 * c * BL for c in PAIRC]   # trees per (group, side)
GNODE = [t * K for t in GTREES]        # nodes per (group, side)
GBASE = [0] * NGRP
for _g in range(1, NGRP):
    GBASE[_g] = GBASE[_g - 1] + GNODE[_g - 1]
assert GBASE[-1] + GNODE[-1] == NL

# ---------------------------------------------------------------------------
# Tree level layout (level-major, recursive even/odd split ordering).
# sigma[l] = heap positions of level l, ordered so that the children of
# sigma[l][i] are sigma[l+1][i] (left) and sigma[l+1][i + n_l] (right).
# ---------------------------------------------------------------------------
_SIGMA = [[0]]
for _l in range(1, 6):
    _prev = _SIGMA[-1]
    _SIGMA.append([2 * p + 1 for p in _prev] + [2 * p + 2 for p in _prev])
_SIGMA.append([63])  # level 6: only heap node 63 (= left child of 31)
_NLVL = [len(s) for s in _SIGMA]            # [1, 2, 4, 8, 16, 32, 1]


def _offsets(gt: int) -> list:
    off = [0]
    for n in _NLVL:
        off.append(off[-1] + gt * n)
    return off


_OFFS = [_offsets(2 * t) for t in GTREES]  # sides merged: 2x trees per group

# subtree sizes per heap position
_SZS = [0] * K
for _j in range(K - 1, -1, -1):
    _SZS[_j] = 1
    if 2 * _j + 1 < K:
        _SZS[_j] += _SZS[2 * _j + 1]
    if 2 * _j + 2 < K:
        _SZS[_j] += _SZS[2 * _j + 2]


def _group_stmts(g: int) -> list:
    lo = [SG[g] + i for i in range(PAIRC[g])]
    hi0 = S - SG[g] - PAIRC[g]
    hi = [hi0 + i for i in range(PAIRC[g])]
    return lo + hi


def _group_perm(g: int) -> np.ndarray:
    """perm[col] = (side, within-core node index) for level-major col of
    group g; sides interleaved at the lane level (t_local = si*16+side*8+p).
    Encoded as side * NL + node."""
    stmts = _group_stmts(g)
    order = []
    for lvl in range(7):
        for t_local in range(2 * GTREES[g]):
            si, side, p = t_local // 16, (t_local // 8) % 2, t_local % 8
            tree_global = p * S + stmts[si]
            for hp in _SIGMA[lvl]:
                order.append(side * NL + tree_global * K + hp)
    return np.asarray(order, dtype=np.int64)


_PERMS = [_group_perm(g) for g in range(NGRP)]


def _szs_levelmajor(gt: int) -> np.ndarray:
    return np.asarray(
        [_SZS[hp] for lvl in range(7) for _t in range(gt) for hp in _SIGMA[lvl]],
        dtype=np.float16,
    )


def _wrap_idx(tokens_perm: np.ndarray) -> np.ndarray:
    """Wrap permuted token list into dma_gather idx layout
    [128, n//16] int16 (idx j -> row j%16, col j//16, replicated x8)."""
    n = len(tokens_perm)
    out = np.zeros((128, n // 16), dtype=np.int16)
    blk = tokens_perm.reshape(n // 16, 16).T.astype(np.int16)
    for grp in range(8):
        out[grp * 16 : (grp + 1) * 16, :] = blk
    return out


# ---------------------------------------------------------------------------
# Kernel builder
# ---------------------------------------------------------------------------

def build_nc() -> bass.Bass:
    nc = bacc.Bacc("TRN2", target_bir_lowering=False)

    emb16 = nc.declare_dram_parameter("emb16", [V, E], FP16, isOutput=False)
    idx_ext = [
        nc.declare_dram_parameter(f"idx{g}", [128, 2 * GNODE[g] // 16], I16, isOutput=False)
        for g in range(NGRP)
    ]
    wcT_ext = nc.declare_dram_parameter("wcT", [E, E], FP16, isOutput=False)
    wih_ext = nc.declare_dram_parameter("wih", [2, E, 3 * E], FP16, isOutput=False)
    whh_ext = nc.declare_dram_parameter("whh", [2, E, 3 * E], FP16, isOutput=False)
    bgi_ext = nc.declare_dram_parameter("bgi", [2, 1, 3 * E], FP16, isOutput=False)
    ghcst_ext = nc.declare_dram_parameter("ghcst", [E, S * 32], FP16, isOutput=False)
    wout_ext = nc.declare_dram_parameter("wout", [E, 1], FP16, isOutput=False)
    bout_ext = nc.declare_dram_parameter("bout", [1, 1], FP32, isOutput=False)
    out_ext = nc.declare_dram_parameter("out", [BL], FP32, isOutput=True)

    with tile.TileContext(nc) as tc, ExitStack() as ctx:
        persist = ctx.enter_context(tc.tile_pool(name="persist", bufs=1))
        work = ctx.enter_context(tc.tile_pool(name="work", bufs=2))

        # ---- prologue: constants + weights -----------------------------
        ident = persist.tile([128, 128], FP16)
        masks.make_identity(nc, ident[:])
        ones = persist.tile([1, 64], FP16)
        nc.vector.memset(ones[:], 1.0)
        # touch every activation function once so table switches happen now
        actwarm = persist.tile([1, 4], FP32)
        nc.scalar.activation(actwarm[:], ones[0:1, 0:4], ACTF.Sigmoid)
        nc.scalar.activation(actwarm[:], ones[0:1, 0:4], ACTF.Tanh)
        nc.scalar.activation(actwarm[:], ones[0:1, 0:4], ACTF.Abs)
        nc.scalar.activation(actwarm[:], ones[0:1, 0:4], ACTF.Identity)

        idx_sb = []
        for i in range(NGRP):
            it = persist.tile([128, 2 * GNODE[i] // 16], I16, name=f"idx{i}")
            nc.sync.dma_start(out=it[:], in_=idx_ext[i][:, :])
            idx_sb.append(it)

        wcT = persist.tile([E, E], FP16)
        nc.sync.dma_start(out=wcT[:], in_=wcT_ext[:, :])
        wih = {}
        whh = {}
        bgi = {}
        for di, d in enumerate(("f", "b")):
            wih[d] = persist.tile([E, 3 * E], FP16, name=f"wih_{d}")
            nc.sync.dma_start(out=wih[d][:], in_=wih_ext[di])
            whh[d] = persist.tile([E, 3 * E], FP16, name=f"whh_{d}")
            nc.sync.dma_start(out=whh[d][:], in_=whh_ext[di])
            bgi[d] = persist.tile([1, 3 * E], FP16, name=f"bgi_{d}")
            nc.sync.dma_start(out=bgi[d][:], in_=bgi_ext[di])
        wout = persist.tile([E, 1], FP16)
        nc.sync.dma_start(out=wout[:], in_=wout_ext[:, :])
        bout = persist.tile([1, 1], FP32)
        nc.sync.dma_start(out=bout[:], in_=bout_ext[:, :])

        # GRU state + precomputed-input buffers
        # girz step block (96 cols): [rf rb znf znb ghnf ghnb]
        girz = persist.tile([128, S * 96], FP16)
        girz_r = girz[:].rearrange("p (k c) -> p k c", c=96)
        nc.sync.dma_start(
            out=girz_r[:, :, 64:96],
            in_=ghcst_ext.rearrange("p (k c) -> p k c", c=32),
        )
        gin = persist.tile([128, S * 32], FP16)   # step block: nf(16) nb(16)
        gin_r = gin[:].rearrange("p (k c) -> p k c", c=32)
        seq2 = persist.tile([128, S * 32], FP16)  # col = k*32 + d*16 + side*8 + prog
        seq2_r = seq2[:].rearrange("p (k d l) -> p k d l", d=2, l=16)
        h_all = persist.tile([128, 32], FP16)     # [hf(16) hb(16)], lane = side*8+prog
        nc.vector.memset(h_all[:], 0.0)

        gT = persist.tile([128, 2 * NL], FP16)

        warmpool = ctx.enter_context(tc.tile_pool(name="warm", bufs=1, space="PSUM"))
        warm_t = warmpool.tile([128, 512], FP32)

        def pe_warm(n=1):
            for _ in range(n):
                nc.tensor.matmul(
                    warm_t[:], ident[:], gT[:, 0:512],
                    start=True, stop=True, skip_group_check=True,
                )

        ppool = ctx.enter_context(tc.tile_pool(name="proj", bufs=3, space="PSUM"))
        gipool = ctx.enter_context(tc.tile_pool(name="gip", bufs=2, space="PSUM"))
        sppool = ctx.enter_context(tc.tile_pool(name="stepp", bufs=2, space="PSUM"))


        # ---------------- tree phase for one (group, side) ----------------
        def gather_group(g: int):
            n = 2 * GNODE[g]
            reg = gT[:, 2 * GBASE[g] : 2 * GBASE[g] + n]
            OFF = _OFFS[g]
            if g < NGRP:
                # L5+L6 land first so the subtree sums can start at
                # half-gather; then the rest of the levels.
                split = (OFF[5] // 128) * 128
                for a, b in ((split, OFF[7]), (OFF[0], split)):
                    w = b - a
                    nc.gpsimd.dma_gather(
                        reg[:, a:b].rearrange("p (o n) -> p o n", o=1),
                        emb16[:, :], idx_sb[g][:, a // 16 : b // 16], w, w, E,
                        transpose=True, single_packet=False,
                    )
            else:
                nc.gpsimd.dma_gather(
                    reg.rearrange("p (o n) -> p o n", o=1),
                    emb16[:, :], idx_sb[g][:], n, n, E,
                    transpose=True, single_packet=False,
                )

        def sums_group(g: int):
            OFF = _OFFS[g]
            GT = 2 * GTREES[g]
            reg = gT[:, 2 * GBASE[g] : 2 * GBASE[g] + 2 * GNODE[g]]

            # subtree sums, level-synchronous, in place (fp16, SBUF only)
            seng = nc.gpsimd if g >= 2 else nc.vector
            l5 = reg[:, OFF[5] : OFF[6]].rearrange("p (t n) -> p t n", n=32)
            seng.tensor_tensor(
                out=l5[:, :, 0:1], in0=l5[:, :, 0:1],
                in1=reg[:, OFF[6] : OFF[7]].rearrange("p (t n) -> p t n", n=1),
                op=ALU.add,
            )
            tmp = work.tile([128, 2 * max(GTREES) * 16], FP16, tag="tmp")
            for lvl in range(4, -1, -1):
                n_l = _NLVL[lvl]
                fd = GT * n_l
                child = reg[:, OFF[lvl + 1] : OFF[lvl + 2]].rearrange(
                    "p (t n) -> p t n", n=2 * n_l
                )
                tv = tmp[:, 0:fd].rearrange("p (t n) -> p t n", n=n_l)
                seng.tensor_tensor(
                    out=tv, in0=child[:, :, 0:n_l], in1=child[:, :, n_l : 2 * n_l],
                    op=ALU.add,
                )
                seng.tensor_tensor(
                    out=reg[:, OFF[lvl] : OFF[lvl] + fd],
                    in0=reg[:, OFF[lvl] : OFF[lvl] + fd],
                    in1=tmp[:, 0:fd],
                    op=ALU.add,
                )

        def folds_group(g: int):
            OFF = _OFFS[g]
            GT = 2 * GTREES[g]
            reg = gT[:, 2 * GBASE[g] : 2 * GBASE[g] + 2 * GNODE[g]]
            veng = nc.gpsimd if g >= 1 else nc.vector

            # projection chunks + bias + max folds (PSUM read once:
            # Act seeds the L5 left halves, DVE does all PSUM max-folds)
            def proj_chunk(ranges, mmax=512):
                width = sum(b - a for a, b in ranges)
                ps = ppool.tile([128, width], FP32, tag="proj")
                c0 = 0
                for a, b in ranges:
                    while a < b:
                        w = min(b - a, mmax)
                        nc.tensor.matmul(
                            ps[:, c0 : c0 + w], wcT[:], reg[:, a : a + w],
                            start=True, stop=True,
                        )
                        a += w
                        c0 += w
                return ps

            mx0 = work.tile([128, 2 * max(GTREES) * 16], FP16, tag="mx0")
            mx1 = work.tile([128, 2 * max(GTREES) * 8], FP16, tag="mx1")
            mx2 = work.tile([128, 2 * max(GTREES) * 4], FP16, tag="mx2")
            mx3 = work.tile([128, 2 * max(GTREES) * 2], FP16, tag="mx3")
            mx4 = work.tile([128, 2 * max(GTREES)], FP16, tag="mx4")

            veng = nc.gpsimd if g >= 1 else nc.vector

            def halve(dst, src, n):
                nc.vector.tensor_tensor(
                    out=dst.rearrange("p (t n) -> p t n", n=n // 2),
                    in0=src[:, :, 0 : n // 2], in1=src[:, :, n // 2 : n],
                    op=ALU.max,
                )

            # L5 -> mx0 (chunks of <=512)
            nl5 = GT * 32 // 512
            for c in range(nl5):
                ps = proj_chunk([(OFF[5] + c * 512, OFF[5] + (c + 1) * 512)])
                pv = ps[:].rearrange("p (t n) -> p t n", n=32)
                mv = mx0[:, c * 256 : (c + 1) * 256].rearrange("p (t n) -> p t n", n=16)
                nc.scalar.activation(mv, pv[:, :, 0:16], ACTF.Identity)
                nc.vector.tensor_tensor(out=mv, in0=mv, in1=pv[:, :, 16:32], op=ALU.max)
            # L4 fold into mx0 (chunks of <=512)
            nl4 = max(1, GT * 16 // 512)
            w4 = GT * 16 // nl4
            for c in range(nl4):
                ps = proj_chunk([(OFF[4] + c * w4, OFF[4] + (c + 1) * w4)])
                nc.vector.tensor_tensor(
                    out=mx0[:, c * w4 : (c + 1) * w4],
                    in0=mx0[:, c * w4 : (c + 1) * w4],
                    in1=ps[:], op=ALU.max,
                )
            halve(mx1[:, 0 : GT * 8], mx0[:, 0 : GT * 16].rearrange("p (t n) -> p t n", n=16), 16)
            # L3 (chunks of <=512)
            nl3 = max(1, GT * 8 // 512)
            w3 = GT * 8 // nl3
            for c in range(nl3):
                ps = proj_chunk([(OFF[3] + c * w3, OFF[3] + (c + 1) * w3)])
                nc.vector.tensor_tensor(
                    out=mx1[:, c * w3 : (c + 1) * w3],
                    in0=mx1[:, c * w3 : (c + 1) * w3], in1=ps[:], op=ALU.max,
                )
            halve(mx2[:, 0 : GT * 4], mx1[:, 0 : GT * 8].rearrange("p (t n) -> p t n", n=8), 8)
            if GT * 4 >= 512:
                # big merged group: L2 alone, then L0+L1+L6 packed
                ps2 = proj_chunk([(OFF[2], OFF[3])])
                nc.vector.tensor_tensor(
                    out=mx2[:, 0 : GT * 4], in0=mx2[:, 0 : GT * 4], in1=ps2[:],
                    op=ALU.max,
                )
                ps = proj_chunk([(OFF[0], OFF[2]), (OFF[6], OFF[7])])
                l1o, l6o = GT, 3 * GT
            else:
                ps = proj_chunk([(OFF[0], OFF[3]), (OFF[6], OFF[7])])
                nc.vector.tensor_tensor(
                    out=mx2[:, 0 : GT * 4], in0=mx2[:, 0 : GT * 4],
                    in1=ps[:, OFF[2] : OFF[3]], op=ALU.max,
                )
                l1o, l6o = OFF[1], OFF[3]
            halve(mx3[:, 0 : GT * 2], mx2[:, 0 : GT * 4].rearrange("p (t n) -> p t n", n=4), 4)
            nc.vector.tensor_tensor(
                out=mx3[:, 0 : GT * 2], in0=mx3[:, 0 : GT * 2],
                in1=ps[:, l1o : l1o + GT * 2], op=ALU.max,
            )
            halve(mx4[:, 0:GT], mx3[:, 0 : GT * 2].rearrange("p (t n) -> p t n", n=2), 2)
            nc.vector.tensor_tensor(
                out=mx4[:, 0:GT], in0=mx4[:, 0:GT], in1=ps[:, 0:GT],
                op=ALU.max,
            )
            nc.vector.tensor_tensor(
                out=mx4[:, 0:GT], in0=mx4[:, 0:GT],
                in1=ps[:, l6o : l6o + GT], op=ALU.max,
            )

            # relu + scatter into seq2 (fwd slot k=stmt, bwd slot k=31-stmt)
            cg = PAIRC[g]
            lanes = slice(0, 16)
            in_lo = mx4[:, 0 : cg * 16].rearrange("p (si r) -> p si r", r=16)
            in_hi = mx4[:, cg * 16 : 2 * cg * 16].rearrange("p (si r) -> p si r", r=16)
            lo0, hi0 = SG[g], S - SG[g] - cg

            def relu_to(out_view, in_view):
                veng.tensor_scalar(
                    out=out_view, in0=in_view, scalar1=0.0, scalar2=None,
                    op0=ALU.max,
                )

            relu_to(seq2_r[:, lo0 : lo0 + cg, 0, lanes], in_lo)
            relu_to(seq2_r[:, hi0 : hi0 + cg, 0, lanes], in_hi)
            relu_to(seq2_r[:, hi0 : hi0 + cg, 1, lanes][:, ::-1, :], in_lo)
            relu_to(seq2_r[:, lo0 : lo0 + cg, 1, lanes][:, ::-1, :], in_hi)

        # ---------------- Gi matmuls for one group ------------------------
        def gi_group(g: int):
            cg = PAIRC[g]
            w = cg * 16
            for k0 in (SG[g], S - SG[g] - cg):
                gp = gipool.tile([128, 384], FP32, tag="gi")
                for di, d in enumerate(("f", "b")):
                    for gate in range(3):
                        c0 = gate * 128 + di * 64
                        nc.tensor.matmul(
                            gp[:, c0 : c0 + w],
                            wih[d][:, gate * 128 : (gate + 1) * 128],
                            seq2_r[:, k0 : k0 + cg, di, :],
                            start=True, stop=False,
                        )
                        nc.tensor.matmul(
                            gp[:, c0 : c0 + w],
                            bgi[d][0:1, gate * 128 : (gate + 1) * 128],
                            ones[:, 0:w],
                            start=False, stop=True,
                        )
                # evac r/zn -> girz (one op per dir), n -> gin
                gp_v = gp[:, 0:256].rearrange(
                    "p (gt d x) -> p gt d x", gt=2, d=2
                )[:, :, :, 0:w].rearrange("p gt d (k j) -> p k gt d j", j=16)
                gz_v = girz_r[:, k0 : k0 + cg, 0:64].rearrange(
                    "p k (gt dj) -> p k gt dj", dj=32
                )
                for di in range(2):
                    nc.scalar.activation(
                        gz_v[:, :, :, di * 16 : di * 16 + 16],
                        gp_v[:, :, :, di, :],
                        ACTF.Identity,
                    )
                for di in range(2):
                    nc.scalar.activation(
                        gin_r[:, k0 : k0 + cg, di * 16 : di * 16 + 16],
                        gp[:, 256 + di * 64 : 256 + di * 64 + w].rearrange(
                            "p (k j) -> p k j", j=16
                        ),
                        ACTF.Identity,
                    )

        # ---------------- one fused GRU step ------------------------------
        def gru_step(k: int):
            late = k >= 2
            veng = nc.gpsimd if late else nc.vector
            ps = sppool.tile([128, 96], FP32, tag="step")
            nc.tensor.matmul(
                ps[:], ident[:], girz[:, k * 96 : (k + 1) * 96],
                start=True, stop=False, skip_group_check=True,
            )
            for gate in range(3):  # r, -z, n
                for di, d in enumerate(("f", "b")):
                    c0 = gate * 32 + di * 16
                    nc.tensor.matmul(
                        ps[:, c0 : c0 + 16],
                        whh[d][:, gate * 128 : (gate + 1) * 128],
                        h_all[:, di * 16 : (di + 1) * 16],
                        start=False, stop=True, skip_group_check=True,
                    )
            rz = work.tile([128, 64], FP32, tag="rz")
            nc.scalar.activation(rz[:], ps[:, 0:64], ACTF.Sigmoid)  # [r, 1-z]
            nmul = work.tile([128, 32], FP16, tag="nmul")
            nc.vector.tensor_tensor(
                out=nmul[:], in0=ps[:, 64:96], in1=rz[:, 0:32], op=ALU.mult
            )
            ninp = work.tile([128, 32], FP16, tag="ninp")
            nc.vector.tensor_tensor(
                out=ninp[:], in0=nmul[:], in1=gin[:, k * 32 : (k + 1) * 32], op=ALU.add
            )
            n_t = work.tile([128, 32], FP16, tag="nt")
            nc.scalar.activation(n_t[:], ninp[:], ACTF.Tanh)
            # h' = h + (1-z) * (n - h)
            t1 = work.tile([128, 32], FP16, tag="t1")
            veng.tensor_tensor(out=t1[:], in0=n_t[:], in1=h_all[:], op=ALU.subtract)
            t2 = work.tile([128, 32], FP16, tag="t2")
            veng.tensor_tensor(out=t2[:], in0=t1[:], in1=rz[:, 32:64], op=ALU.mult)
            veng.tensor_tensor(out=h_all[:], in0=t2[:], in1=h_all[:], op=ALU.add)
            pe_warm(2)

        # ---------------- emission ----------------------------------------
        for g in range(3):
            gather_group(g)
        for g in range(NGRP):
            sums_group(g)
            pe_warm(4)
            folds_group(g)
            gi_group(g)
            for k in range(SG[g], SG[g] + PAIRC[g]):
                gru_step(k)
            if 2 <= g < NGRP - 1:
                gather_group(g + 1)
        for k in range(SG[-1] + PAIRC[-1], S):
            gru_step(k)

        # ---- head: sigmoid(|l - r| @ w_out.T + b_out) --------------------
        hs = work.tile([128, 16], FP32, tag="hs")
        nc.vector.tensor_tensor(
            out=hs[:], in0=h_all[:, 0:16], in1=h_all[:, 16:32], op=ALU.add
        )
        d0 = work.tile([128, 8], FP32, tag="d0")
        nc.vector.tensor_tensor(
            out=d0[:], in0=hs[:, 0:8], in1=hs[:, 8:16], op=ALU.subtract
        )
        dabs = work.tile([128, 8], FP16, tag="dabs")
        nc.scalar.activation(dabs[:], d0[:], ACTF.Abs)
        po = ppool.tile([1, 8], FP32, tag="proj")
        nc.tensor.matmul(po[:], wout[:], dabs[:], start=True, stop=True)
        osb = work.tile([1, 8], FP32, tag="osb")
        nc.scalar.activation(osb[:], po[:], ACTF.Sigmoid, bias=bout[:])
        nc.sync.dma_start(out=out_ext.rearrange("(o j) -> o j", o=1), in_=osb[:])

    nc.compile()
    return nc


_NC_CACHE = None


def _get_nc():
    global _NC_CACHE
    if _NC_CACHE is None:
        _NC_CACHE = build_nc()
    return _NC_CACHE


def make_in_maps(inputs: dict) -> list:
    """Host-side prep: shard + permute tokens, convert/transpose weights."""
    tokens = [
        np.asarray(inputs["tokens1"]).astype(np.int64),
        np.asarray(inputs["tokens2"]).astype(np.int64),
    ]
    w_c = np.asarray(inputs["w_c"], np.float64)
    b_c = np.asarray(inputs["b_c"], np.float64)
    # fold the projection bias into the table: W_c @ (emb + t) = W_c @ emb + b_c
    try:
        t_bias = np.linalg.solve(w_c, b_c)
    except np.linalg.LinAlgError:
        t_bias = np.linalg.lstsq(w_c, b_c, rcond=None)[0]
    emb16 = (np.asarray(inputs["emb"], np.float64) + t_bias[None, :]).astype(np.float16)
    b_hh = {d: np.asarray(inputs[f"b_hh_{d}"], np.float32) for d in ("f", "b")}
    b_ih = {d: np.asarray(inputs[f"b_ih_{d}"], np.float32) for d in ("f", "b")}

    ghcst = np.zeros((E, S * 32), np.float16)
    ghv = ghcst.reshape(E, S, 32)
    ghv[:, :, 0:16] = b_hh["f"][256:384][:, None, None]
    ghv[:, :, 16:32] = b_hh["b"][256:384][:, None, None]

    def stack3(wt):  # [384, 128] -> [128, 384] blocks [r, -z, n] transposed
        t = wt.T  # [128, 384]
        return np.concatenate([t[:, 0:128], -t[:, 128:256], t[:, 256:384]], axis=1)

    def bgi_rows(d):  # [1, 384]: bsum_r, -bsum_z, b_ih_n
        br = b_ih[d][0:128] + b_hh[d][0:128]
        bz = b_ih[d][128:256] + b_hh[d][128:256]
        return np.concatenate([br, -bz, b_ih[d][256:384]])[None, :].astype(np.float16)

    rep = {
        "emb16": emb16,
        "wcT": np.ascontiguousarray(w_c.T).astype(np.float16),
        "wih": np.stack(
            [np.ascontiguousarray(stack3(np.asarray(inputs[f"w_ih_{d}"], np.float32))).astype(np.float16) for d in ("f", "b")]
        ),
        "whh": np.stack(
            [np.ascontiguousarray(stack3(np.asarray(inputs[f"w_hh_{d}"], np.float32))).astype(np.float16) for d in ("f", "b")]
        ),
        "bgi": np.stack([bgi_rows(d) for d in ("f", "b")]),
        "ghcst": ghcst,
        "wout": np.ascontiguousarray(np.asarray(inputs["w_out"], np.float32).T).astype(np.float16),
        "bout": np.asarray(inputs["b_out"], np.float32).reshape(1, 1),
    }
    in_maps = []
    for i in range(NCORES):
        m = dict(rep)
        both = np.concatenate(
            [tokens[0][i * NL : (i + 1) * NL], tokens[1][i * NL : (i + 1) * NL]]
        )
        for g in range(NGRP):
            m[f"idx{g}"] = _wrap_idx(both[_PERMS[g]])
        in_maps.append(m)
    return in_maps


def kernel(**inputs) -> np.ndarray:
    nc = _get_nc()
    in_maps = make_in_maps(inputs)
    res = run_bass_kernel_spmd(nc, in_maps, list(range(NCORES)))
    out = np.concatenate(
        [np.asarray(res.results[i]["out"], np.float32).reshape(BL, 1) for i in range(NCORES)],
        axis=0,
    )
    return out


# revision 72
# speedup vs baseline: 2.7191x; 1.0090x over previous
"""Trainium2 Bass kernel for nn_BatchProgramCC (tree-GRU program classifier).

Contract: kernel(**inputs) takes FULL unsharded inputs (numpy), returns the
FULL [64, 1] float32 output. Internally shards the B=64 program batch over
8 NeuronCores (8 programs / core), runs one SPMD Bass program, gathers.

Hardcoded problem shape (from the reference):
  V=30000 vocab, E=ENC=H=128, B=64, S=32 statements, K=64 nodes/tree,
  heap tree layout (parent of j is (j-1)//2 within each 64-node block),
  LEVELS=8 level-sync iterations == exact subtree sums (depth 6).

Pipeline per core (per side, 16384 nodes):
  - transpose-mode dma_gather straight from the fp16 embedding table in
    DRAM -> gT[E, node] in SBUF, node columns in level-major order,
    both sides merged, split into 5 two-ended statement groups (pair
    counts 2,4,4,4,2) so
    the fused fwd+bwd GRU can start early and finish soon after the
    last gather.
  - level-synchronous subtree sums in-place on gT (fp16; pair-sums on
    Pool, accumulate on DVE).
  - projection W_c @ S per <=512-col chunk on PE (fp16), + b_c x size
    rank-1 matmul into the same PSUM; max-folding reads PSUM once
    (Act seeds + DVE folds; GPSIMD cannot touch PSUM on HW).
  - relu + scatter into per-(step, dir) GRU input layout (Pool).
  - fused fwd+bwd GRU, 3 gates (r, 1-z, n; z-weights sign-folded on the
    host) so the blend is h' = h + (1-z)*(n - h).
  - chained idle matmuls keep the PE p-state ramped so projection
    matmuls run at full clock.
"""

import numpy as np
from contextlib import ExitStack

try:
    import concourse.bass as bass
except ImportError:  # pragma: no cover
    import sys

    sys.path.insert(0, "/opt/trn_rl_repo")
    import concourse.bass as bass

import concourse.bacc as bacc
import concourse.mybir as mybir
import concourse.tile as tile
from concourse import masks
from concourse.bass_utils import run_bass_kernel_spmd

FP32 = mybir.dt.float32
FP16 = mybir.dt.float16
I16 = mybir.dt.int16
ALU = mybir.AluOpType
ACTF = mybir.ActivationFunctionType

V = 30000
E = 128          # embedding/encode/hidden dim (all 128)
NCORES = 8
BL = 8           # programs per core
S = 32           # statements (GRU steps)
K = 64           # nodes per tree
NL = BL * S * K  # nodes per side per core = 16384

PAIRC = [2, 4, 4, 4, 2]          # statement pairs per two-ended group
NGRP = len(PAIRC)
SG = [0] * NGRP                  # first step of each group
for _g in range(1, NGRP):
    SG[_g] = SG[_g - 1] + PAIRC[_g - 1]
GTREES = [2 * c * BL for c in PAIRC]   # trees per (group, side)
GNODE = [t * K for t in GTREES]        # nodes per (group, side)
GBASE = [0] * NGRP
for _g in range(1, NGRP):
    GBASE[_g] = GBASE[_g - 1] + GNODE[_g - 1]
assert GBASE[-1] + GNODE[-1] == NL

# ---------------------------------------------------------------------------
# Tree level layout (level-major, recursive even/odd split ordering).
# sigma[l] = heap positions of level l, ordered so that the children of
# sigma[l][i] are sigma[l+1][i] (left) and sigma[l+1][i + n_l] (right).
# ---------------------------------------------------------------------------
_SIGMA = [[0]]
for _l in range(1, 6):
    _prev = _SIGMA[-1]
    _SIGMA.append([2 * p + 1 for p in _prev] + [2 * p + 2 for p in _prev])
_SIGMA.append([63])  # level 6: only heap node 63 (= left child of 31)
_NLVL = [len(s) for s in _SIGMA]            # [1, 2, 4, 8, 16, 32, 1]


def _offsets(gt: int) -> list:
    off = [0]
    for n in _NLVL:
        off.append(off[-1] + gt * n)
    return off


_OFFS = [_offsets(2 * t) for t in GTREES]  # sides merged: 2x trees per group

# subtree sizes per heap position
_SZS = [0] * K
for _j in range(K - 1, -1, -1):
    _SZS[_j] = 1
    if 2 * _j + 1 < K:
        _SZS[_j] += _SZS[2 * _j + 1]
    if 2 * _j + 2 < K:
        _SZS[_j] += _SZS[2 * _j + 2]


def _group_stmts(g: int) -> list:
    lo = [SG[g] + i for i in range(PAIRC[g])]
    hi0 = S - SG[g] - PAIRC[g]
    hi = [hi0 + i for i in range(PAIRC[g])]
    return lo + hi


def _group_perm(g: int) -> np.ndarray:
    """perm[col] = (side, within-core node index) for level-major col of
    group g; sides interleaved at the lane level (t_local = si*16+side*8+p).
    Encoded as side * NL + node."""
    stmts = _group_stmts(g)
    order = []
    for lvl in range(7):
        for t_local in range(2 * GTREES[g]):
            si, side, p = t_local // 16, (t_local // 8) % 2, t_local % 8
            tree_global = p * S + stmts[si]
            for hp in _SIGMA[lvl]:
                order.append(side * NL + tree_global * K + hp)
    return np.asarray(order, dtype=np.int64)


_PERMS = [_group_perm(g) for g in range(NGRP)]


def _szs_levelmajor(gt: int) -> np.ndarray:
    return np.asarray(
        [_SZS[hp] for lvl in range(7) for _t in range(gt) for hp in _SIGMA[lvl]],
        dtype=np.float16,
    )


def _wrap_idx(tokens_perm: np.ndarray) -> np.ndarray:
    """Wrap permuted token list into dma_gather idx layout
    [128, n//16] int16 (idx j -> row j%16, col j//16, replicated x8)."""
    n = len(tokens_perm)
    out = np.zeros((128, n // 16), dtype=np.int16)
    blk = tokens_perm.reshape(n // 16, 16).T.astype(np.int16)
    for grp in range(8):
        out[grp * 16 : (grp + 1) * 16, :] = blk
    return out


# ---------------------------------------------------------------------------
# Kernel builder
# ---------------------------------------------------------------------------

def build_nc() -> bass.Bass:
    nc = bacc.Bacc("TRN2", target_bir_lowering=False)

    emb16 = nc.declare_dram_parameter("emb16", [V, E], FP16, isOutput=False)
    idx_ext = [
        nc.declare_dram_parameter(f"idx{g}", [128, 2 * GNODE[g] // 16], I16, isOutput=False)
        for g in range(NGRP)
    ]
    wcT_ext = nc.declare_dram_parameter("wcT", [E, E], FP16, isOutput=False)
    wih_ext = nc.declare_dram_parameter("wih", [2, E, 3 * E], FP16, isOutput=False)
    whh_ext = nc.declare_dram_parameter("whh", [2, E, 3 * E], FP16, isOutput=False)
    bgi_ext = nc.declare_dram_parameter("bgi", [2, 1, 3 * E], FP16, isOutput=False)
    ghcst_ext = nc.declare_dram_parameter("ghcst", [E, S * 32], FP16, isOutput=False)
    wout_ext = nc.declare_dram_parameter("wout", [E, 1], FP16, isOutput=False)
    bout_ext = nc.declare_dram_parameter("bout", [1, 1], FP32, isOutput=False)
    out_ext = nc.declare_dram_parameter("out", [BL], FP32, isOutput=True)

    with tile.TileContext(nc) as tc, ExitStack() as ctx:
        persist = ctx.enter_context(tc.tile_pool(name="persist", bufs=1))
        work = ctx.enter_context(tc.tile_pool(name="work", bufs=2))

        # ---- prologue: constants + weights -----------------------------
        ident = persist.tile([128, 128], FP16)
        masks.make_identity(nc, ident[:])
        ones = persist.tile([1, 64], FP16)
        nc.vector.memset(ones[:], 1.0)
        # touch every activation function once so table switches happen now
        actwarm = persist.tile([1, 4], FP32)
        nc.scalar.activation(actwarm[:], ones[0:1, 0:4], ACTF.Sigmoid)
        nc.scalar.activation(actwarm[:], ones[0:1, 0:4], ACTF.Tanh)
        nc.scalar.activation(actwarm[:], ones[0:1, 0:4], ACTF.Abs)
        nc.scalar.activation(actwarm[:], ones[0:1, 0:4], ACTF.Identity)

        idx_sb = []
        for i in range(NGRP):
            it = persist.tile([128, 2 * GNODE[i] // 16], I16, name=f"idx{i}")
            nc.sync.dma_start(out=it[:], in_=idx_ext[i][:, :])
            idx_sb.append(it)

        wcT = persist.tile([E, E], FP16)
        nc.sync.dma_start(out=wcT[:], in_=wcT_ext[:, :])
        wih = {}
        whh = {}
        bgi = {}
        for di, d in enumerate(("f", "b")):
            wih[d] = persist.tile([E, 3 * E], FP16, name=f"wih_{d}")
            nc.sync.dma_start(out=wih[d][:], in_=wih_ext[di])
            whh[d] = persist.tile([E, 3 * E], FP16, name=f"whh_{d}")
            nc.sync.dma_start(out=whh[d][:], in_=whh_ext[di])
            bgi[d] = persist.tile([1, 3 * E], FP16, name=f"bgi_{d}")
            nc.sync.dma_start(out=bgi[d][:], in_=bgi_ext[di])
        wout = persist.tile([E, 1], FP16)
        nc.sync.dma_start(out=wout[:], in_=wout_ext[:, :])
        bout = persist.tile([1, 1], FP32)
        nc.sync.dma_start(out=bout[:], in_=bout_ext[:, :])

        # GRU state + precomputed-input buffers
        # girz step block (96 cols): [rf rb znf znb ghnf ghnb]
        girz = persist.tile([128, S * 96], FP16)
        girz_r = girz[:].rearrange("p (k c) -> p k c", c=96)
        nc.sync.dma_start(
            out=girz_r[:, :, 64:96],
            in_=ghcst_ext.rearrange("p (k c) -> p k c", c=32),
        )
        gin = persist.tile([128, S * 32], FP16)   # step block: nf(16) nb(16)
        gin_r = gin[:].rearrange("p (k c) -> p k c", c=32)
        seq2 = persist.tile([128, S * 32], FP16)  # col = k*32 + d*16 + side*8 + prog
        seq2_r = seq2[:].rearrange("p (k d l) -> p k d l", d=2, l=16)
        h_all = persist.tile([128, 32], FP16)     # [hf(16) hb(16)], lane = side*8+prog
        nc.vector.memset(h_all[:], 0.0)

        gT = persist.tile([128, 2 * NL], FP16)

        warmpool = ctx.enter_context(tc.tile_pool(name="warm", bufs=1, space="PSUM"))
        warm_t = warmpool.tile([128, 512], FP32)

        def pe_warm(n=1):
            for _ in range(n):
                nc.tensor.matmul(
                    warm_t[:], ident[:], gT[:, 0:512],
                    start=True, stop=True, skip_group_check=True,
                )

        ppool = ctx.enter_context(tc.tile_pool(name="proj", bufs=3, space="PSUM"))
        gipool = ctx.enter_context(tc.tile_pool(name="gip", bufs=2, space="PSUM"))
        sppool = ctx.enter_context(tc.tile_pool(name="stepp", bufs=2, space="PSUM"))


        # ---------------- tree phase for one (group, side) ----------------
        def gather_group(g: int):
            n = 2 * GNODE[g]
            reg = gT[:, 2 * GBASE[g] : 2 * GBASE[g] + n]
            OFF = _OFFS[g]
            if g < NGRP:
                # L5+L6 land first so the subtree sums can start at
                # half-gather; then the rest of the levels.
                s5 = (OFF[5] // 128) * 128
                s4 = (OFF[4] // 128) * 128
                s3 = (OFF[3] // 128) * 128
                for a, b in ((s5, OFF[7]), (s4, s5), (s3, s4), (OFF[0], s3)):
                    w = b - a
                    nc.gpsimd.dma_gather(
                        reg[:, a:b].rearrange("p (o n) -> p o n", o=1),
                        emb16[:, :], idx_sb[g][:, a // 16 : b // 16], w, w, E,
                        transpose=True, single_packet=False,
                    )
            else:
                nc.gpsimd.dma_gather(
                    reg.rearrange("p (o n) -> p o n", o=1),
                    emb16[:, :], idx_sb[g][:], n, n, E,
                    transpose=True, single_packet=False,
                )

        def sums_group(g: int):
            OFF = _OFFS[g]
            GT = 2 * GTREES[g]
            reg = gT[:, 2 * GBASE[g] : 2 * GBASE[g] + 2 * GNODE[g]]

            # subtree sums, level-synchronous, in place (fp16, SBUF only)
            seng = nc.gpsimd if g >= 2 else nc.vector
            l5 = reg[:, OFF[5] : OFF[6]].rearrange("p (t n) -> p t n", n=32)
            seng.tensor_tensor(
                out=l5[:, :, 0:1], in0=l5[:, :, 0:1],
                in1=reg[:, OFF[6] : OFF[7]].rearrange("p (t n) -> p t n", n=1),
                op=ALU.add,
            )
            tmp = work.tile([128, 2 * max(GTREES) * 16], FP16, tag="tmp")
            for lvl in range(4, -1, -1):
                n_l = _NLVL[lvl]
                fd = GT * n_l
                child = reg[:, OFF[lvl + 1] : OFF[lvl + 2]].rearrange(
                    "p (t n) -> p t n", n=2 * n_l
                )
                tv = tmp[:, 0:fd].rearrange("p (t n) -> p t n", n=n_l)
                seng.tensor_tensor(
                    out=tv, in0=child[:, :, 0:n_l], in1=child[:, :, n_l : 2 * n_l],
                    op=ALU.add,
                )
                seng.tensor_tensor(
                    out=reg[:, OFF[lvl] : OFF[lvl] + fd],
                    in0=reg[:, OFF[lvl] : OFF[lvl] + fd],
                    in1=tmp[:, 0:fd],
                    op=ALU.add,
                )

        def folds_group(g: int):
            OFF = _OFFS[g]
            GT = 2 * GTREES[g]
            reg = gT[:, 2 * GBASE[g] : 2 * GBASE[g] + 2 * GNODE[g]]
            veng = nc.gpsimd if g >= 1 else nc.vector

            # projection chunks + bias + max folds (PSUM read once:
            # Act seeds the L5 left halves, DVE does all PSUM max-folds)
            def proj_chunk(ranges, mmax=512):
                width = sum(b - a for a, b in ranges)
                ps = ppool.tile([128, width], FP32, tag="proj")
                c0 = 0
                for a, b in ranges:
                    while a < b:
                        w = min(b - a, mmax)
                        nc.tensor.matmul(
                            ps[:, c0 : c0 + w], wcT[:], reg[:, a : a + w],
                            start=True, stop=True,
                        )
                        a += w
                        c0 += w
                return ps

            mx0 = work.tile([128, 2 * max(GTREES) * 16], FP16, tag="mx0")
            mx1 = work.tile([128, 2 * max(GTREES) * 8], FP16, tag="mx1")
            mx2 = work.tile([128, 2 * max(GTREES) * 4], FP16, tag="mx2")
            mx3 = work.tile([128, 2 * max(GTREES) * 2], FP16, tag="mx3")
            mx4 = work.tile([128, 2 * max(GTREES)], FP16, tag="mx4")

            veng = nc.gpsimd if g >= 1 else nc.vector

            def halve(dst, src, n):
                nc.vector.tensor_tensor(
                    out=dst.rearrange("p (t n) -> p t n", n=n // 2),
                    in0=src[:, :, 0 : n // 2], in1=src[:, :, n // 2 : n],
                    op=ALU.max,
                )

            # L5 -> mx0 (chunks of <=512)
            nl5 = GT * 32 // 512
            for c in range(nl5):
                ps = proj_chunk([(OFF[5] + c * 512, OFF[5] + (c + 1) * 512)])
                pv = ps[:].rearrange("p (t n) -> p t n", n=32)
                mv = mx0[:, c * 256 : (c + 1) * 256].rearrange("p (t n) -> p t n", n=16)
                nc.scalar.activation(mv, pv[:, :, 0:16], ACTF.Identity)
                nc.vector.tensor_tensor(out=mv, in0=mv, in1=pv[:, :, 16:32], op=ALU.max)
            # L4 fold into mx0 (chunks of <=512)
            nl4 = max(1, GT * 16 // 512)
            w4 = GT * 16 // nl4
            for c in range(nl4):
                ps = proj_chunk([(OFF[4] + c * w4, OFF[4] + (c + 1) * w4)])
                nc.vector.tensor_tensor(
                    out=mx0[:, c * w4 : (c + 1) * w4],
                    in0=mx0[:, c * w4 : (c + 1) * w4],
                    in1=ps[:], op=ALU.max,
                )
            halve(mx1[:, 0 : GT * 8], mx0[:, 0 : GT * 16].rearrange("p (t n) -> p t n", n=16), 16)
            # L3 (chunks of <=512)
            nl3 = max(1, GT * 8 // 512)
            w3 = GT * 8 // nl3
            for c in range(nl3):
                ps = proj_chunk([(OFF[3] + c * w3, OFF[3] + (c + 1) * w3)])
                nc.vector.tensor_tensor(
                    out=mx1[:, c * w3 : (c + 1) * w3],
                    in0=mx1[:, c * w3 : (c + 1) * w3], in1=ps[:], op=ALU.max,
                )
            halve(mx2[:, 0 : GT * 4], mx1[:, 0 : GT * 8].rearrange("p (t n) -> p t n", n=8), 8)
            if GT * 4 >= 512:
                # big merged group: L2 alone, then L0+L1+L6 packed
                ps2 = proj_chunk([(OFF[2], OFF[3])])
                nc.vector.tensor_tensor(
                    out=mx2[:, 0 : GT * 4], in0=mx2[:, 0 : GT * 4], in1=ps2[:],
                    op=ALU.max,
                )
                ps = proj_chunk([(OFF[0], OFF[2]), (OFF[6], OFF[7])])
                l1o, l6o = GT, 3 * GT
            else:
                ps = proj_chunk([(OFF[0], OFF[3]), (OFF[6], OFF[7])])
                nc.vector.tensor_tensor(
                    out=mx2[:, 0 : GT * 4], in0=mx2[:, 0 : GT * 4],
                    in1=ps[:, OFF[2] : OFF[3]], op=ALU.max,
                )
                l1o, l6o = OFF[1], OFF[3]
            halve(mx3[:, 0 : GT * 2], mx2[:, 0 : GT * 4].rearrange("p (t n) -> p t n", n=4), 4)
            nc.vector.tensor_tensor(
                out=mx3[:, 0 : GT * 2], in0=mx3[:, 0 : GT * 2],
                in1=ps[:, l1o : l1o + GT * 2], op=ALU.max,
            )
            halve(mx4[:, 0:GT], mx3[:, 0 : GT * 2].rearrange("p (t n) -> p t n", n=2), 2)
            nc.vector.tensor_tensor(
                out=mx4[:, 0:GT], in0=mx4[:, 0:GT], in1=ps[:, 0:GT],
                op=ALU.max,
            )
            nc.vector.tensor_tensor(
                out=mx4[:, 0:GT], in0=mx4[:, 0:GT],
                in1=ps[:, l6o : l6o + GT], op=ALU.max,
            )

            # relu + scatter into seq2 (fwd slot k=stmt, bwd slot k=31-stmt)
            cg = PAIRC[g]
            lanes = slice(0, 16)
            in_lo = mx4[:, 0 : cg * 16].rearrange("p (si r) -> p si r", r=16)
            in_hi = mx4[:, cg * 16 : 2 * cg * 16].rearrange("p (si r) -> p si r", r=16)
            lo0, hi0 = SG[g], S - SG[g] - cg

            def relu_to(out_view, in_view):
                veng.tensor_scalar(
                    out=out_view, in0=in_view, scalar1=0.0, scalar2=None,
                    op0=ALU.max,
                )

            relu_to(seq2_r[:, lo0 : lo0 + cg, 0, lanes], in_lo)
            relu_to(seq2_r[:, hi0 : hi0 + cg, 0, lanes], in_hi)
            relu_to(seq2_r[:, hi0 : hi0 + cg, 1, lanes][:, ::-1, :], in_lo)
            relu_to(seq2_r[:, lo0 : lo0 + cg, 1, lanes][:, ::-1, :], in_hi)

        # ---------------- Gi matmuls for one group ------------------------
        def gi_group(g: int):
            cg = PAIRC[g]
            w = cg * 16
            for k0 in (SG[g], S - SG[g] - cg):
                gp = gipool.tile([128, 384], FP32, tag="gi")
                for di, d in enumerate(("f", "b")):
                    for gate in range(3):
                        c0 = gate * 128 + di * 64
                        nc.tensor.matmul(
                            gp[:, c0 : c0 + w],
                            wih[d][:, gate * 128 : (gate + 1) * 128],
                            seq2_r[:, k0 : k0 + cg, di, :],
                            start=True, stop=False,
                        )
                        nc.tensor.matmul(
                            gp[:, c0 : c0 + w],
                            bgi[d][0:1, gate * 128 : (gate + 1) * 128],
                            ones[:, 0:w],
                            start=False, stop=True,
                        )
                # evac r/zn -> girz (one op per dir), n -> gin
                gp_v = gp[:, 0:256].rearrange(
                    "p (gt d x) -> p gt d x", gt=2, d=2
                )[:, :, :, 0:w].rearrange("p gt d (k j) -> p k gt d j", j=16)
                gz_v = girz_r[:, k0 : k0 + cg, 0:64].rearrange(
                    "p k (gt dj) -> p k gt dj", dj=32
                )
                for di in range(2):
                    nc.scalar.activation(
                        gz_v[:, :, :, di * 16 : di * 16 + 16],
                        gp_v[:, :, :, di, :],
                        ACTF.Identity,
                    )
                for di in range(2):
                    nc.scalar.activation(
                        gin_r[:, k0 : k0 + cg, di * 16 : di * 16 + 16],
                        gp[:, 256 + di * 64 : 256 + di * 64 + w].rearrange(
                            "p (k j) -> p k j", j=16
                        ),
                        ACTF.Identity,
                    )

        # ---------------- one fused GRU step ------------------------------
        def gru_step(k: int):
            late = k >= 2
            veng = nc.gpsimd if late else nc.vector
            ps = sppool.tile([128, 96], FP32, tag="step")
            nc.tensor.matmul(
                ps[:], ident[:], girz[:, k * 96 : (k + 1) * 96],
                start=True, stop=False, skip_group_check=True,
            )
            for gate in range(3):  # r, -z, n
                for di, d in enumerate(("f", "b")):
                    c0 = gate * 32 + di * 16
                    nc.tensor.matmul(
                        ps[:, c0 : c0 + 16],
                        whh[d][:, gate * 128 : (gate + 1) * 128],
                        h_all[:, di * 16 : (di + 1) * 16],
                        start=False, stop=True, skip_group_check=True,
                    )
            rz = work.tile([128, 64], FP32, tag="rz")
            nc.scalar.activation(rz[:], ps[:, 0:64], ACTF.Sigmoid)  # [r, 1-z]
            nmul = work.tile([128, 32], FP16, tag="nmul")
            nc.vector.tensor_tensor(
                out=nmul[:], in0=ps[:, 64:96], in1=rz[:, 0:32], op=ALU.mult
            )
            ninp = work.tile([128, 32], FP16, tag="ninp")
            nc.vector.tensor_tensor(
                out=ninp[:], in0=nmul[:], in1=gin[:, k * 32 : (k + 1) * 32], op=ALU.add
            )
            n_t = work.tile([128, 32], FP16, tag="nt")
            nc.scalar.activation(n_t[:], ninp[:], ACTF.Tanh)
            # h' = h + (1-z) * (n - h)
            t1 = work.tile([128, 32], FP16, tag="t1")
            veng.tensor_tensor(out=t1[:], in0=n_t[:], in1=h_all[:], op=ALU.subtract)
            t2 = work.tile([128, 32], FP16, tag="t2")
            veng.tensor_tensor(out=t2[:], in0=t1[:], in1=rz[:, 32:64], op=ALU.mult)
            veng.tensor_tensor(out=h_all[:], in0=t2[:], in1=h_all[:], op=ALU.add)
            pe_warm(2)

        # ---------------- emission ----------------------------------------
        for g in range(3):
            gather_group(g)
        for g in range(NGRP):
            sums_group(g)
            pe_warm(4)
            folds_group(g)
            gi_group(g)
            for k in range(SG[g], SG[g] + PAIRC[g]):
                gru_step(k)
            if 2 <= g < NGRP - 1:
                gather_group(g + 1)
        for k in range(SG[-1] + PAIRC[-1], S):
            gru_step(k)

        # ---- head: sigmoid(|l - r| @ w_out.T + b_out) --------------------
        hs = work.tile([128, 16], FP32, tag="hs")
        nc.vector.tensor_tensor(
            out=hs[:], in0=h_all[:, 0:16], in1=h_all[:, 16:32], op=ALU.add
        )
        d0 = work.tile([128, 8], FP32, tag="d0")
        nc.vector.tensor_tensor(
            out=d0[:], in0=hs[:, 0:8], in1=hs[:, 8:16], op=ALU.subtract
        )
        dabs = work.tile([128, 8], FP16, tag="dabs")
        nc.scalar.activation(dabs[:], d0[:], ACTF.Abs)
        po = ppool.tile([1, 8], FP32, tag="proj")
        nc.tensor.matmul(po[:], wout[:], dabs[:], start=True, stop=True)
        osb = work.tile([1, 8], FP32, tag="osb")
        nc.scalar.activation(osb[:], po[:], ACTF.Sigmoid, bias=bout[:])
        nc.sync.dma_start(out=out_ext.rearrange("(o j) -> o j", o=1), in_=osb[:])

    nc.compile()
    return nc


_NC_CACHE = None


def _get_nc():
    global _NC_CACHE
    if _NC_CACHE is None:
        _NC_CACHE = build_nc()
    return _NC_CACHE


def make_in_maps(inputs: dict) -> list:
    """Host-side prep: shard + permute tokens, convert/transpose weights."""
    tokens = [
        np.asarray(inputs["tokens1"]).astype(np.int64),
        np.asarray(inputs["tokens2"]).astype(np.int64),
    ]
    w_c = np.asarray(inputs["w_c"], np.float64)
    b_c = np.asarray(inputs["b_c"], np.float64)
    # fold the projection bias into the table: W_c @ (emb + t) = W_c @ emb + b_c
    try:
        t_bias = np.linalg.solve(w_c, b_c)
    except np.linalg.LinAlgError:
        t_bias = np.linalg.lstsq(w_c, b_c, rcond=None)[0]
    emb16 = (np.asarray(inputs["emb"], np.float64) + t_bias[None, :]).astype(np.float16)
    b_hh = {d: np.asarray(inputs[f"b_hh_{d}"], np.float32) for d in ("f", "b")}
    b_ih = {d: np.asarray(inputs[f"b_ih_{d}"], np.float32) for d in ("f", "b")}

    ghcst = np.zeros((E, S * 32), np.float16)
    ghv = ghcst.reshape(E, S, 32)
    ghv[:, :, 0:16] = b_hh["f"][256:384][:, None, None]
    ghv[:, :, 16:32] = b_hh["b"][256:384][:, None, None]

    def stack3(wt):  # [384, 128] -> [128, 384] blocks [r, -z, n] transposed
        t = wt.T  # [128, 384]
        return np.concatenate([t[:, 0:128], -t[:, 128:256], t[:, 256:384]], axis=1)

    def bgi_rows(d):  # [1, 384]: bsum_r, -bsum_z, b_ih_n
        br = b_ih[d][0:128] + b_hh[d][0:128]
        bz = b_ih[d][128:256] + b_hh[d][128:256]
        return np.concatenate([br, -bz, b_ih[d][256:384]])[None, :].astype(np.float16)

    rep = {
        "emb16": emb16,
        "wcT": np.ascontiguousarray(w_c.T).astype(np.float16),
        "wih": np.stack(
            [np.ascontiguousarray(stack3(np.asarray(inputs[f"w_ih_{d}"], np.float32))).astype(np.float16) for d in ("f", "b")]
        ),
        "whh": np.stack(
            [np.ascontiguousarray(stack3(np.asarray(inputs[f"w_hh_{d}"], np.float32))).astype(np.float16) for d in ("f", "b")]
        ),
        "bgi": np.stack([bgi_rows(d) for d in ("f", "b")]),
        "ghcst": ghcst,
        "wout": np.ascontiguousarray(np.asarray(inputs["w_out"], np.float32).T).astype(np.float16),
        "bout": np.asarray(inputs["b_out"], np.float32).reshape(1, 1),
    }
    in_maps = []
    for i in range(NCORES):
        m = dict(rep)
        both = np.concatenate(
            [tokens[0][i * NL : (i + 1) * NL], tokens[1][i * NL : (i + 1) * NL]]
        )
        for g in range(NGRP):
            m[f"idx{g}"] = _wrap_idx(both[_PERMS[g]])
        in_maps.append(m)
    return in_maps


def kernel(**inputs) -> np.ndarray:
    nc = _get_nc()
    in_maps = make_in_maps(inputs)
    res = run_bass_kernel_spmd(nc, in_maps, list(range(NCORES)))
    out = np.concatenate(
        [np.asarray(res.results[i]["out"], np.float32).reshape(BL, 1) for i in range(NCORES)],
        axis=0,
    )
    return out
